# revision 46
# baseline (speedup 1.0000x reference)
"""VQ codebook kernel (nn_KW_CascadedBranch) for 8 Trainium2 NeuronCores.

Reference computation:
    kw   = audio_feat @ proj_w + proj_b                  [B,N,512]
    cos  = normalize(kw) @ normalize(token_embedding).T  [B,N,V]
    p    = softmax(cos / 0.1)
    out  = p @ token_embedding                           [B,N,512]

Strategy: tensor-parallel over the vocab dim V=49408. Each core owns a
6176-row shard (padded to 6400 = 50*128 = 25 DoubleRow pairs), keeps both
embedding layouts resident in SBUF as fp8e4, and computes the partial
(p @ emb) plus the partial softmax denominator for ALL B*N=2048 slots.
Softmax needs no max subtraction: logits = 10*cos are in [-10,10].
Host combines the 8 partials: out = (sum_c pe_c)/64 / (sum_c d_c).

The two big GEMMs (cos-scores, prob@emb) run on the PE in fp8e4 with
MatmulPerfMode.DoubleRow: two 128-deep contraction slices per instruction
at 0.5 cycles/row, i.e. 4x the fp32r rate. Quantization scales are powers
of two folded into the exp scale and the host epilogue:
  kwn*32, emb_n*32 -> scores_psum = 1024*cos, exp scale = 10/1024,
  emb*64           -> out_pe = 64*numerator.
The projection runs in bf16 (its quantization noise is amplified ~40x by
the flat-softmax cancellation, so fp8 is not enough there), with proj_b
folded in as a rank-1 row of the same PSUM accumulation group. Embedding
row norms are host-precomputed weight prep; vocab-pad rows are zeros so
exp(0)=1 there, and the denominator matmul uses a masked ones stationary
(>=32 columns, a dual-fp8 ldweights requirement) to exclude them exactly.

fp8 error feedback: out is a near-cancelling average over ~40k vocab rows
(|out|_rms ~ sigma_emb/200), so the kwn fp8 rounding error delta couples
through J = 10*Cov_p(emb, emb_n) into an output error ~40x larger than
naive estimates. Softmax here is nearly flat (den/VS ~= exp(T^-2/(2*512))
uniformly, +-0.7% over m), so J is well approximated with flat weights:
delta_num ~= dbar*10*S^T@delta, with S = sum_shard emb emb_n^T a host
constant and dbar the spec-derived density constant. The kernel captures
delta (the fp8 rounding residual, written straight to fp8 during the
transpose copy) and adds q8(20*dbar*S^T) @ q8(res32) into the same kwacc
PSUM accumulation group: 2 extra DoubleRow matmuls per (j, mc).
Validated in numpy + HW: maxrel 3.4e-2 -> 8.2e-3.

Scheduling: engine queues are in-order, so the 16-m-tile projection
prologue is split: tiles 0-3 run up front in a 4-bank pipelined scope
(closed before the main pools open), tiles 4-15 are emitted inside the
mc0/mc1 kk loops through a single shared PSUM bank, filling the PE's
slack under the ACT-bound exp stream. GEMM1 for iteration kk+1 issues
ahead of GEMM2(kk) so the PE queue never waits on the exps.
"""

import numpy as np
import ml_dtypes

import concourse.bass as bass
import concourse.mybir as mybir
from concourse import tile
from concourse.bass_utils import run_bass_kernel_spmd

F32 = mybir.dt.float32
F8 = mybir.dt.float8e4
BF16 = mybir.dt.bfloat16
AF = mybir.ActivationFunctionType
OP = mybir.AluOpType
PM = mybir.MatmulPerfMode
NPF8 = ml_dtypes.float8_e4m3
NPBF = ml_dtypes.bfloat16

N_CORES = 8
B, N, D, E, V = 256, 8, 768, 512, 49408
M = B * N                      # 2048 keyword slots
VS = V // N_CORES              # 6176 real vocab rows per core
VT = 50                        # v-tiles of 128 per core (6400 rows, 224 pad)
VP = VT * 128
KK = VT // 2                   # 25 DoubleRow v-tile pairs
MC = 512                       # m-chunk (columns per PSUM accumulator)
NMC = M // MC                  # 4
MT = M // 128                  # 16 m-tiles in the projection prologue
DT = D // 128                  # 6 d-chunks
EC = E // 128                  # 4 e-chunks (2 DoubleRow pairs)
EXP_SCALE = 10.0 / 1024.0      # 1/T divided by the 32*32 quant scales
W_SCALE = 64.0                 # emb quant scale
N_SCALE = 32.0                 # kwn / emb_n quant scale
DBAR_R = float(np.exp(100.0 / 1024.0))  # E[den]/VS for unit-norm randn data

# (mc, kk) -> prologue m-tile emitted at that point of the main loop
PRO_SCHED = {}
UPFRONT = 16


def _split_multiwait_ctrl(nc, max_waits: int = 1) -> int:
    """This container's walrus rejects instructions carrying more than one
    semaphore wait (CTRL and S3_LW encodings alike). Hoist overflow waits
    onto same-engine NoOps inserted immediately before the offender."""
    n_split = 0
    for fn in nc.m.functions:
        for bb in fn.blocks:
            rebuilt, changed = [], False
            for ins in bb.instructions:
                si = ins.sync_info
                if (
                    si is not None
                    and si.on_wait
                    and len(si.on_wait) > max_waits
                ):
                    waits = list(si.on_wait)
                    head, tail = waits[:-max_waits], waits[-max_waits:]
                    for i in range(0, len(head), max_waits):
                        nop = mybir.InstNoOp(name=f"{ins.name}-ws{i}", ins=[], outs=[])
                        nop.engine = ins.engine
                        nop.sync_info = mybir.SyncInfo(
                            on_wait=head[i:i + max_waits], on_update=[]
                        )
                        rebuilt.append(nop)
                    ins.sync_info = mybir.SyncInfo(
                        on_wait=tail, on_update=list(si.on_update or [])
                    )
                    changed = True
                    n_split += 1
                rebuilt.append(ins)
            if changed:
                bb.instructions = rebuilt
    return n_split


def _pair(ap2d, width):
    """View a flat [128, 2*width] AP as [128, 2, width]."""
    return ap2d.rearrange("p (a w) -> p a w", a=2, w=width)


def build_program():
    nc = bass.Bass(target_bir_lowering=False)

    audio_t = nc.dram_tensor("audio_t", [D, M], BF16, kind="ExternalInput")
    pw_t = nc.dram_tensor("pw_t", [D, E], BF16, kind="ExternalInput")
    proj_b = nc.dram_tensor("proj_b", [1, E], BF16, kind="ExternalInput")
    et2 = nc.dram_tensor("et2", [2, 128, 2 * VP], F8, kind="ExternalInput")
    en2 = nc.dram_tensor("en2", [128, KK * 2 * E], F8, kind="ExternalInput")
    k2 = nc.dram_tensor("k2", [2, 128, 2 * E], F8, kind="ExternalInput")
    onesv = nc.dram_tensor("onesv", [128, 128], F8, kind="ExternalInput")
    ident = nc.dram_tensor("ident", [128, 128], BF16, kind="ExternalInput")

    out_pe = nc.dram_tensor("out_pe", [E, M], F32, kind="ExternalOutput")
    out_d = nc.dram_tensor("out_d", [1, M], F32, kind="ExternalOutput")

    with tile.TileContext(nc) as tc:
        with (
            tc.tile_pool(name="resident", bufs=1) as res,
            tc.tile_pool(name="small", bufs=1) as small,
            tc.tile_pool(name="prok", bufs=6) as prok,
        ):
            # ---- resident SBUF tiles; DMAs ordered so compute starts early:
            # pw+audio chunks (prologue) -> et2 slices (GEMM1) -> en2 (GEMM2)
            pw_all = res.tile([128, DT * E], BF16, tag="pw_all")
            pb_sb = small.tile([1, E], BF16, tag="pb")
            id_sb = small.tile([128, 128], BF16, tag="ident")
            a_all = res.tile([128, DT * M], BF16, tag="a_all")
            et_all = res.tile([128, 4 * VP], F8, tag="et_all")
            ones_sb = small.tile([128, 128], F8, tag="ones_sb")
            en_sb = res.tile([128, KK * 2 * E], F8, tag="en")
            k_all = res.tile([128, 4 * E], F8, tag="k_all")

            et3 = [
                _pair(et_all[:, j * 2 * VP:(j + 1) * 2 * VP], VP) for j in range(2)
            ]
            a4 = a_all[:].rearrange("p (d m) -> p d m", d=DT, m=M)
            a4s = audio_t[:].rearrange("(d p) m -> p d m", d=DT, p=128)
            etd = [_pair(et2[j], VP) for j in range(2)]
            k4 = k_all[:].rearrange("p (j x) -> p j x", j=2, x=2 * E)
            k4s = k2[:].rearrange("j p x -> p j x")
            # interleave the streams, fewest DMAs (each costs ~300ns of queue
            # overhead): audio chunk 0 + first et quarter feed the upfront
            # prologue and GEMM1(kk=0); the correction stationary and the
            # first en piece land before the kk=0 GEMM2 group opens; the
            # trailing et quarters and en pieces arrive mid-loop
            QW = VP // 4
            ENP = (KK * 2 * E) // 5
            nc.sync.dma_start(
                pw_all[:].rearrange("p (d e) -> p d e", d=DT, e=E),
                pw_t[:].rearrange("(d p) e -> p d e", d=DT, p=128),
            )
            nc.sync.dma_start(pb_sb[:], proj_b[:])
            nc.sync.dma_start(id_sb[:], ident[:])
            def a_tile_dma(lo, hi):
                sl = slice(lo * 128, hi * 128)
                nc.sync.dma_start(a4[:, :, sl], a4s[:, :, sl])

            a_tile_dma(0, 4)
            for j in range(2):
                nc.sync.dma_start(et3[j][:, :, 0:QW], etd[j][:, :, 0:QW])
            nc.sync.dma_start(ones_sb[:], onesv[:])
            nc.sync.dma_start(k4[:], k4s[:])
            nc.sync.dma_start(en_sb[:, 0:ENP], en2[:, 0:ENP])
            a_tile_dma(4, 6)
            for j in range(2):
                nc.sync.dma_start(et3[j][:, :, QW:2 * QW], etd[j][:, :, QW:2 * QW])
            a_tile_dma(6, 9)
            nc.sync.dma_start(en_sb[:, ENP:2 * ENP], en2[:, ENP:2 * ENP])
            for j in range(2):
                nc.sync.dma_start(et3[j][:, :, 2 * QW:3 * QW], etd[j][:, :, 2 * QW:3 * QW])
            a_tile_dma(9, 12)
            for j in range(2):
                nc.sync.dma_start(et3[j][:, :, 3 * QW:4 * QW], etd[j][:, :, 3 * QW:4 * QW])
            a_tile_dma(12, 16)
            for pc in range(2, 5):
                sl = slice(pc * ENP, (pc + 1) * ENP)
                nc.sync.dma_start(en_sb[:, sl], en2[:, sl])
            kwnT = [
                [
                    res.tile([128, 2 * MC], F8, tag=f"kwnT{j}_{c}", name=f"kwnT{j}_{c}")
                    for c in range(NMC)
                ]
                for j in range(2)
            ]
            res8 = [
                [
                    res.tile([128, 2 * MC], F8, tag=f"res8_{j}_{c}", name=f"res8_{j}_{c}")
                    for c in range(NMC)
                ]
                for j in range(2)
            ]
            ones_row = small.tile([1, 128], BF16, tag="ones_row")
            nc.vector.memset(ones_row[:], 1.0)

            kwnT3 = [[_pair(kwnT[j][c][:], MC) for c in range(NMC)] for j in range(2)]
            res8_3 = [[_pair(res8[j][c][:], MC) for c in range(NMC)] for j in range(2)]
            k3 = [_pair(k_all[:, j * 2 * E:(j + 1) * 2 * E], E) for j in range(2)]
            ones3 = _pair(ones_sb[:], 64)

            MAGIC = 0x5F3759DF

            def pro_tile(i, kw_alloc, tp_alloc, act_norm):
                """Projection + normalize + transpose + fp8/residual capture
                for m-tile i. kw_alloc/tp_alloc hand out PSUM tiles. The
                upfront tiles use ACT Square+Sqrt (ACT is idle at start); the
                in-loop tiles keep ACT exp-only and compute 32*rsqrt on DVE
                via the 0x5f3759df bit trick + two Newton steps."""
                kw_ps = kw_alloc(i)
                for d in range(DT):
                    nc.tensor.matmul(
                        kw_ps[:],
                        a_all[:, d * M + i * 128:d * M + (i + 1) * 128],
                        pw_all[:, d * E:(d + 1) * E],
                        start=(d == 0), stop=False,
                    )
                nc.tensor.matmul(
                    kw_ps[:], ones_row[:], pb_sb[:], start=False, stop=True
                )
                sq = prok.tile([128, E], F32, tag="sq_kw", name=f"sq{i}")
                nsq = prok.tile([128, 1], F32, tag="nsq_kw", name=f"nsq{i}")
                tk = prok.tile([128, 1], F32, tag="tk", name=f"tk{i}")
                if act_norm:
                    nc.scalar.activation(sq[:], kw_ps[:], AF.Square, accum_out=nsq[:])
                    sk = prok.tile([128, 1], F32, tag="sk", name=f"sk{i}")
                    nc.scalar.activation(sk[:], nsq[:], AF.Sqrt)
                    rk = prok.tile([128, 1], F32, tag="rk", name=f"rk{i}")
                    nc.vector.reciprocal(rk[:], sk[:])
                    nc.vector.tensor_mul(tk[:], rk[:], rk[:])
                    nc.vector.tensor_mul(tk[:], tk[:], nsq[:])
                    nc.vector.tensor_scalar(tk[:], tk[:], -16.0, 48.0, OP.mult, OP.add)
                    nc.vector.tensor_mul(tk[:], tk[:], rk[:])
                else:
                    nc.vector.tensor_mul(sq[:], kw_ps[:], kw_ps[:])
                    nc.vector.tensor_reduce(nsq[:], sq[:], mybir.AxisListType.X, OP.add)
                    t1 = prok.tile([128, 1], mybir.dt.int32, tag="t1", name=f"t1_{i}")
                    nc.vector.tensor_scalar(
                        t1[:], nsq[:].bitcast(mybir.dt.int32), 1, 0,
                        OP.logical_shift_right, OP.logical_shift_right,
                    )
                    y0 = prok.tile([128, 1], mybir.dt.int32, tag="y0", name=f"y0_{i}")
                    nc.vector.tensor_scalar(y0[:], t1[:], -1, MAGIC, OP.mult, OP.add)
                    hs = prok.tile([128, 1], F32, tag="hs", name=f"hs{i}")
                    nc.vector.tensor_scalar(hs[:], nsq[:], 0.5, 0.0, OP.mult, OP.add)
                    ya = y0[:].bitcast(F32)
                    aa = prok.tile([128, 1], F32, tag="aa", name=f"aa{i}")
                    cc = prok.tile([128, 1], F32, tag="cc", name=f"cc{i}")
                    y1 = prok.tile([128, 1], F32, tag="y1", name=f"y1_{i}")
                    nc.vector.tensor_mul(aa[:], ya, ya)
                    nc.vector.tensor_mul(aa[:], aa[:], hs[:])
                    nc.vector.tensor_scalar(cc[:], aa[:], -1.0, 1.5, OP.mult, OP.add)
                    nc.vector.tensor_mul(y1[:], ya, cc[:])
                    nc.vector.tensor_mul(aa[:], y1[:], y1[:])
                    nc.vector.tensor_mul(aa[:], aa[:], hs[:])
                    nc.vector.tensor_scalar(cc[:], aa[:], -32.0, 48.0, OP.mult, OP.add)
                    nc.vector.tensor_mul(tk[:], y1[:], cc[:])
                kwn = prok.tile([128, E], BF16, tag="kwn", name=f"kwn{i}")
                nc.vector.tensor_scalar_mul(kwn[:], kw_ps[:], tk[:])
                for j in range(EC):
                    tpv = tp_alloc(i, j)
                    nc.tensor.transpose(tpv, kwn[:, j * 128:(j + 1) * 128], id_sb[:])
                    c2 = (j % 2) * MC + (i % 4) * 128
                    kpiece = kwnT[j // 2][i // 4][:, c2:c2 + 128]
                    nc.vector.tensor_copy(kpiece, tpv)
                    # fp8 rounding residual (32-scale) for the J-correction
                    nc.vector.tensor_sub(
                        res8[j // 2][i // 4][:, c2:c2 + 128], tpv, kpiece
                    )

            # ---- upfront prologue: m-tiles 0..3 in a pipelined 4-bank scope
            with (
                tc.tile_pool(name="pro_ps", bufs=2, space="PSUM") as pro_ps,
                tc.tile_pool(name="pro_ps2", bufs=2, space="PSUM") as pro_ps2,
            ):
                def kw_up(i):
                    return pro_ps.tile([128, E], F32, tag="kw_ps", name=f"kwps{i}")

                def tp_up(i, j):
                    t = pro_ps2.tile([128, 128], BF16, tag="tp", name=f"tp{i}_{j}")
                    return t[:]

                for i in range(UPFRONT):
                    pro_tile(i, kw_up, tp_up, act_norm=True)

            # ---- main loop. m-tiles 4..15 stream in through one PSUM bank
            # during mc0/mc1; that bank becomes a third score buffer for
            # mc2/mc3 (the exp->GEMM1 bank-recycle latency costs ~15% of the
            # exp pace at depth 2).
            with (
                tc.tile_pool(name="acc_ps", bufs=4, space="PSUM") as acc_ps,
                tc.tile_pool(name="d_ps", bufs=1, space="PSUM") as d_ps,
                tc.tile_pool(name="pp", bufs=6) as pp,
                tc.tile_pool(name="ob", bufs=8) as ob,
            ):
                def run_mc(mc, sc_ps, pro_in):
                    def kw_in(i):
                        return pro_in.tile([128, E], F32, tag="pro", name=f"kwps{i}")

                    def tp_in(i, j):
                        t = pro_in.tile([128, E], F32, tag="pro", name=f"tp{i}_{j}")
                        return t[:].bitcast(BF16)[:, 0:128]

                    def gemm1(kk):
                        tiles = []
                        for t in range(2):
                            k = 2 * kk + t
                            sc = sc_ps.tile([128, MC], F32, tag="sc", name=f"sc{kk}_{t}")
                            for j in range(2):
                                nc.tensor.matmul(
                                    sc[:],
                                    et3[j][:, :, k * 128:(k + 1) * 128],
                                    kwnT3[j][mc],
                                    start=(j == 0), stop=(j == 1),
                                    perf_mode=PM.DoubleRow,
                                )
                            tiles.append(sc)
                        return tiles

                    m0 = mc * MC
                    kwacc = [
                        acc_ps.tile([128, MC], F32, tag="kwacc", name=f"kwacc{j}")
                        for j in range(EC)
                    ]
                    dacc = d_ps.tile([32, MC], F32, tag="dacc")
                    cur = gemm1(0)
                    for kk in range(KK):
                        p2 = pp.tile([128, 2 * MC], F8, tag="p2")
                        p3 = _pair(p2[:], MC)
                        for t in range(2):
                            nc.scalar.activation(
                                p2[:, t * MC:(t + 1) * MC], cur[t][:], AF.Exp,
                                scale=EXP_SCALE,
                            )
                        # issue next iteration's score GEMMs ahead of GEMM2 so
                        # the in-order PE queue never stalls behind the exps
                        if kk + 1 < KK:
                            cur = gemm1(kk + 1)
                        if kk == 0:
                            # J-correction opens each kwacc group (residuals
                            # are ready before the mc starts; PSUM addition is
                            # order-free), so the mc tail is just the last
                            # GEMM2 + the output copies
                            for j in range(EC):
                                for jj in range(2):
                                    nc.tensor.matmul(
                                        kwacc[j][:],
                                        k3[jj][:, :, j * 128:(j + 1) * 128],
                                        res8_3[jj][mc],
                                        start=(jj == 0), stop=False,
                                        perf_mode=PM.DoubleRow,
                                    )
                        en_kk = _pair(en_sb[:, kk * 2 * E:(kk + 1) * 2 * E], E)
                        for j in range(EC):
                            nc.tensor.matmul(
                                kwacc[j][:],
                                en_kk[:, :, j * 128:(j + 1) * 128],
                                p3,
                                start=False, stop=(kk == KK - 1),
                                perf_mode=PM.DoubleRow,
                            )
                        sel = 32 if kk == KK - 1 else 0
                        nc.tensor.matmul(
                            dacc[:],
                            ones3[:, :, sel:sel + 32],
                            p3,
                            start=(kk == 0), stop=(kk == KK - 1),
                            perf_mode=PM.DoubleRow,
                        )
                        ti = PRO_SCHED.get((mc, kk))
                        if ti is not None:
                            pro_tile(ti, kw_in, tp_in, act_norm=True)
                    dsb = ob.tile([1, MC], F32, tag="dsb")
                    nc.vector.tensor_copy(dsb[:], dacc[0:1, :])
                    nc.sync.dma_start(out_d[:, m0:m0 + MC], dsb[:])
                    for j in range(EC):
                        osb = ob.tile([128, MC], F32, tag="osb")
                        nc.vector.tensor_copy(osb[:], kwacc[j][:])
                        nc.sync.dma_start(
                            out_pe[j * 128:(j + 1) * 128, m0:m0 + MC], osb[:]
                        )

                with tc.tile_pool(name="sc_all", bufs=3, space="PSUM") as sc_all:
                    for mc in range(NMC):
                        run_mc(mc, sc_all, None)
    return nc


_CACHED = {}


def _get_program():
    if "nc" not in _CACHED:
        nc = build_program()
        _split_multiwait_ctrl(nc)
        _CACHED["nc"] = nc
    return _CACHED["nc"]


def _q8(x):
    return np.asarray(x, np.float32).astype(NPF8)


def _prep_in_maps(audio_feat, proj_w, proj_b, token_embedding):
    audio = np.asarray(audio_feat, np.float32).reshape(M, D)
    pw = np.asarray(proj_w, np.float32)
    pb = np.asarray(proj_b, np.float32).reshape(1, E)
    emb = np.asarray(token_embedding, np.float32)

    audio_t = np.ascontiguousarray(audio.T).astype(NPBF)
    pw_t = np.ascontiguousarray(pw).astype(NPBF)
    pb16 = pb.astype(NPBF)
    ident = np.eye(128, dtype=np.float32).astype(NPBF)
    # masked ones columns for the denominator matmul (dual-fp8 ldweights
    # needs >=32 stationary columns): [128, 2, 64] -> cols 0:32 all-real
    # pair, cols 32:64 last pair (tile 48 rows 0..31 real, tile 49 pad)
    nreal_last = VS - (VT - 2) * 128          # 32 real rows in tile 48
    onesv = np.zeros((128, 2, 64), np.float32)
    onesv[:, :, 0:32] = 1.0
    onesv[:nreal_last, 0, 32:64] = 1.0
    onesv = _q8(onesv.reshape(128, 128))

    in_maps = []
    for c in range(N_CORES):
        shard = np.zeros((VP, E), np.float32)
        shard[:VS] = emb[c * VS:(c + 1) * VS]
        nrm = np.maximum(np.linalg.norm(shard, axis=1, keepdims=True), 1e-8)
        emb_n = shard / nrm
        # et2: [jj, 128e, 2, VP] with component i = e-chunk (2*jj+i)
        et = (emb_n.T * N_SCALE).reshape(2, 2, 128, VP).transpose(0, 2, 1, 3)
        et2 = np.ascontiguousarray(_q8(et).reshape(2, 128, 2 * VP))
        # en2: [128v, KK, 2, E] with component i = v-tile (2*kk+i)
        en = (shard * W_SCALE).reshape(KK, 2, 128, E).transpose(2, 0, 1, 3)
        en2 = np.ascontiguousarray(_q8(en).reshape(128, KK * 2 * E))
        # correction stationary: KL[f, e] = q8(20 dbar S^T), S = shard^T emb_n
        S = shard.T @ emb_n                    # [e, f]
        kl = _q8(20.0 * DBAR_R * S.T)          # [f, e]
        k2 = np.ascontiguousarray(
            kl.reshape(2, 2, 128, E).transpose(0, 2, 1, 3).reshape(2, 128, 2 * E)
        )
        in_maps.append({
            "audio_t": audio_t,
            "pw_t": pw_t,
            "proj_b": pb16,
            "et2": et2,
            "en2": en2,
            "k2": k2,
            "onesv": onesv,
            "ident": ident,
        })
    return in_maps


def kernel(audio_feat, proj_w, proj_b, token_embedding, _trace=False):
    nc = _get_program()
    in_maps = _prep_in_maps(audio_feat, proj_w, proj_b, token_embedding)
    res = run_bass_kernel_spmd(
        nc, in_maps, core_ids=list(range(N_CORES)), trace=_trace
    )
    pe = np.zeros((E, M), np.float64)
    dn = np.zeros((1, M), np.float64)
    for c in range(N_CORES):
        pe += res.results[c]["out_pe"]
        dn += res.results[c]["out_d"]
    out = (pe / W_SCALE / dn).T.reshape(B, N, E).astype(np.float32)
    if _trace:
        return out, res
    return out


# revision 49
# speedup vs baseline: 1.0279x; 1.0279x over previous
"""VQ codebook kernel (nn_KW_CascadedBranch) for 8 Trainium2 NeuronCores.

Reference computation:
    kw   = audio_feat @ proj_w + proj_b                  [B,N,512]
    cos  = normalize(kw) @ normalize(token_embedding).T  [B,N,V]
    p    = softmax(cos / 0.1)
    out  = p @ token_embedding                           [B,N,512]

Strategy: tensor-parallel over the vocab dim V=49408. Each core owns a
6176-row shard (padded to 6400 = 50*128 = 25 DoubleRow pairs), keeps both
embedding layouts resident in SBUF as fp8e4, and computes the partial
(p @ emb) plus the partial softmax denominator for ALL B*N=2048 slots.
Softmax needs no max subtraction: logits = 10*cos are in [-10,10].
Host combines the 8 partials: out = (sum_c pe_c)/64 / (sum_c d_c).

The two big GEMMs (cos-scores, prob@emb) run on the PE in fp8e4 with
MatmulPerfMode.DoubleRow: two 128-deep contraction slices per instruction
at 0.5 cycles/row, i.e. 4x the fp32r rate. Quantization scales are powers
of two folded into the exp scale and the host epilogue:
  kwn*32, emb_n*32 -> scores_psum = 1024*cos, exp scale = 10/1024,
  emb*64           -> out_pe = 64*numerator.
The projection runs in bf16 (its quantization noise is amplified ~40x by
the flat-softmax cancellation, so fp8 is not enough there), with proj_b
folded in as a rank-1 row of the same PSUM accumulation group. Embedding
row norms are host-precomputed weight prep; vocab-pad rows are zeros so
exp(0)=1 there, and the denominator matmul uses a masked ones stationary
(>=32 columns, a dual-fp8 ldweights requirement) to exclude them exactly.

fp8 error feedback: out is a near-cancelling average over ~40k vocab rows
(|out|_rms ~ sigma_emb/200), so the kwn fp8 rounding error delta couples
through J = 10*Cov_p(emb, emb_n) into an output error ~40x larger than
naive estimates. Softmax here is nearly flat (den/VS ~= exp(T^-2/(2*512))
uniformly, +-0.7% over m), so J is well approximated with flat weights:
delta_num ~= dbar*10*S^T@delta, with S = sum_shard emb emb_n^T a host
constant and dbar the spec-derived density constant. The kernel captures
delta (the fp8 rounding residual, written straight to fp8 during the
transpose copy) and adds q8(20*dbar*S^T) @ q8(res32) into the same kwacc
PSUM accumulation group: 2 extra DoubleRow matmuls per (j, mc).
Validated in numpy + HW: maxrel 3.4e-2 -> 8.2e-3.

Scheduling: engine queues are in-order, so the 16-m-tile projection
prologue is split: tiles 0-3 run up front in a 4-bank pipelined scope
(closed before the main pools open), tiles 4-15 are emitted inside the
mc0/mc1 kk loops through a single shared PSUM bank, filling the PE's
slack under the ACT-bound exp stream. GEMM1 for iteration kk+1 issues
ahead of GEMM2(kk) so the PE queue never waits on the exps.
"""

import numpy as np
import ml_dtypes

import concourse.bass as bass
import concourse.mybir as mybir
from concourse import tile
from concourse.bass_utils import run_bass_kernel_spmd

F32 = mybir.dt.float32
F8 = mybir.dt.float8e4
BF16 = mybir.dt.bfloat16
AF = mybir.ActivationFunctionType
OP = mybir.AluOpType
PM = mybir.MatmulPerfMode
NPF8 = ml_dtypes.float8_e4m3
NPBF = ml_dtypes.bfloat16

N_CORES = 8
B, N, D, E, V = 256, 8, 768, 512, 49408
M = B * N                      # 2048 keyword slots
VS = V // N_CORES              # 6176 real vocab rows per core
VT = 50                        # v-tiles of 128 per core (6400 rows, 224 pad)
VP = VT * 128
KK = VT // 2                   # 25 DoubleRow v-tile pairs
MC = 512                       # m-chunk (columns per PSUM accumulator)
NMC = M // MC                  # 4
MT = M // 128                  # 16 m-tiles in the projection prologue
DT = D // 128                  # 6 d-chunks
EC = E // 128                  # 4 e-chunks (2 DoubleRow pairs)
EXP_SCALE = 10.0 / 1024.0      # 1/T divided by the 32*32 quant scales
W_SCALE = 64.0                 # emb quant scale
N_SCALE = 32.0                 # kwn / emb_n quant scale
DBAR_R = float(np.exp(100.0 / 1024.0))  # E[den]/VS for unit-norm randn data

# (mc, kk) -> prologue m-tile emitted at that point of the main loop
PRO_SCHED = {}
UPFRONT = 16


def _split_multiwait_ctrl(nc, max_waits: int = 1) -> int:
    """This container's walrus rejects instructions carrying more than one
    semaphore wait (CTRL and S3_LW encodings alike). Hoist overflow waits
    onto same-engine NoOps inserted immediately before the offender."""
    n_split = 0
    for fn in nc.m.functions:
        for bb in fn.blocks:
            rebuilt, changed = [], False
            for ins in bb.instructions:
                si = ins.sync_info
                if (
                    si is not None
                    and si.on_wait
                    and len(si.on_wait) > max_waits
                ):
                    waits = list(si.on_wait)
                    head, tail = waits[:-max_waits], waits[-max_waits:]
                    for i in range(0, len(head), max_waits):
                        nop = mybir.InstNoOp(name=f"{ins.name}-ws{i}", ins=[], outs=[])
                        nop.engine = ins.engine
                        nop.sync_info = mybir.SyncInfo(
                            on_wait=head[i:i + max_waits], on_update=[]
                        )
                        rebuilt.append(nop)
                    ins.sync_info = mybir.SyncInfo(
                        on_wait=tail, on_update=list(si.on_update or [])
                    )
                    changed = True
                    n_split += 1
                rebuilt.append(ins)
            if changed:
                bb.instructions = rebuilt
    return n_split


def _pair(ap2d, width):
    """View a flat [128, 2*width] AP as [128, 2, width]."""
    return ap2d.rearrange("p (a w) -> p a w", a=2, w=width)


def build_program():
    nc = bass.Bass(target_bir_lowering=False)

    audio_t = nc.dram_tensor("audio_t", [D, M], BF16, kind="ExternalInput")
    pw_t = nc.dram_tensor("pw_t", [D, E], BF16, kind="ExternalInput")
    proj_b = nc.dram_tensor("proj_b", [1, E], BF16, kind="ExternalInput")
    et2 = nc.dram_tensor("et2", [2, 128, 2 * VP], F8, kind="ExternalInput")
    en2 = nc.dram_tensor("en2", [128, KK * 2 * E], F8, kind="ExternalInput")
    k2 = nc.dram_tensor("k2", [2, 128, 2 * E], F8, kind="ExternalInput")
    onesv = nc.dram_tensor("onesv", [128, 128], F8, kind="ExternalInput")
    ident = nc.dram_tensor("ident", [128, 128], BF16, kind="ExternalInput")

    out_pe = nc.dram_tensor("out_pe", [E, M], F32, kind="ExternalOutput")
    out_d = nc.dram_tensor("out_d", [1, M], F32, kind="ExternalOutput")

    with tile.TileContext(nc) as tc:
        with (
            tc.tile_pool(name="resident", bufs=1) as res,
            tc.tile_pool(name="small", bufs=1) as small,
            tc.tile_pool(name="prok", bufs=6) as prok,
        ):
            # ---- resident SBUF tiles; DMAs ordered so compute starts early:
            # pw+audio chunks (prologue) -> et2 slices (GEMM1) -> en2 (GEMM2)
            pw_all = res.tile([128, DT * E], BF16, tag="pw_all")
            pb_sb = small.tile([1, E], BF16, tag="pb")
            id_sb = small.tile([128, 128], BF16, tag="ident")
            a_all = res.tile([128, DT * M], BF16, tag="a_all")
            et_all = res.tile([128, 4 * VP], F8, tag="et_all")
            ones_sb = small.tile([128, 128], F8, tag="ones_sb")
            en_sb = res.tile([128, KK * 2 * E], F8, tag="en")
            k_all = res.tile([128, 4 * E], F8, tag="k_all")

            et3 = [
                _pair(et_all[:, j * 2 * VP:(j + 1) * 2 * VP], VP) for j in range(2)
            ]
            a4 = a_all[:].rearrange("p (d m) -> p d m", d=DT, m=M)
            a4s = audio_t[:].rearrange("(d p) m -> p d m", d=DT, p=128)
            etd = [_pair(et2[j], VP) for j in range(2)]
            k4 = k_all[:].rearrange("p (j x) -> p j x", j=2, x=2 * E)
            k4s = k2[:].rearrange("j p x -> p j x")
            # interleave the streams, fewest DMAs (each costs ~300ns of queue
            # overhead): audio chunk 0 + first et quarter feed the upfront
            # prologue and GEMM1(kk=0); the correction stationary and the
            # first en piece land before the kk=0 GEMM2 group opens; the
            # trailing et quarters and en pieces arrive mid-loop
            QW = VP // 4
            ENP = (KK * 2 * E) // 5
            nc.sync.dma_start(
                pw_all[:].rearrange("p (d e) -> p d e", d=DT, e=E),
                pw_t[:].rearrange("(d p) e -> p d e", d=DT, p=128),
            )
            nc.sync.dma_start(pb_sb[:], proj_b[:])
            nc.sync.dma_start(id_sb[:], ident[:])
            def a_tile_dma(lo, hi):
                sl = slice(lo * 128, hi * 128)
                nc.sync.dma_start(a4[:, :, sl], a4s[:, :, sl])

            a_tile_dma(0, 4)
            for j in range(2):
                nc.sync.dma_start(et3[j][:, :, 0:QW], etd[j][:, :, 0:QW])
            nc.sync.dma_start(ones_sb[:], onesv[:])
            nc.sync.dma_start(k4[:], k4s[:])
            nc.sync.dma_start(en_sb[:, 0:ENP], en2[:, 0:ENP])
            a_tile_dma(4, 6)
            for j in range(2):
                nc.sync.dma_start(et3[j][:, :, QW:2 * QW], etd[j][:, :, QW:2 * QW])
            a_tile_dma(6, 9)
            nc.sync.dma_start(en_sb[:, ENP:2 * ENP], en2[:, ENP:2 * ENP])
            for j in range(2):
                nc.sync.dma_start(et3[j][:, :, 2 * QW:3 * QW], etd[j][:, :, 2 * QW:3 * QW])
            a_tile_dma(9, 12)
            for j in range(2):
                nc.sync.dma_start(et3[j][:, :, 3 * QW:4 * QW], etd[j][:, :, 3 * QW:4 * QW])
            a_tile_dma(12, 16)
            for pc in range(2, 5):
                sl = slice(pc * ENP, (pc + 1) * ENP)
                nc.sync.dma_start(en_sb[:, sl], en2[:, sl])
            kwnT = [
                [
                    res.tile([128, 2 * MC], F8, tag=f"kwnT{j}_{c}", name=f"kwnT{j}_{c}")
                    for c in range(NMC)
                ]
                for j in range(2)
            ]
            res8 = [
                [
                    res.tile([128, 2 * MC], F8, tag=f"res8_{j}_{c}", name=f"res8_{j}_{c}")
                    for c in range(NMC)
                ]
                for j in range(2)
            ]
            ones_row = small.tile([1, 128], BF16, tag="ones_row")
            nc.vector.memset(ones_row[:], 1.0)

            kwnT3 = [[_pair(kwnT[j][c][:], MC) for c in range(NMC)] for j in range(2)]
            res8_3 = [[_pair(res8[j][c][:], MC) for c in range(NMC)] for j in range(2)]
            k3 = [_pair(k_all[:, j * 2 * E:(j + 1) * 2 * E], E) for j in range(2)]
            ones3 = _pair(ones_sb[:], 64)

            MAGIC = 0x5F3759DF

            def pro_tile(i, kw_alloc, tp_alloc, act_norm):
                """Projection + normalize + transpose + fp8/residual capture
                for m-tile i. kw_alloc/tp_alloc hand out PSUM tiles. The
                upfront tiles use ACT Square+Sqrt (ACT is idle at start); the
                in-loop tiles keep ACT exp-only and compute 32*rsqrt on DVE
                via the 0x5f3759df bit trick + two Newton steps."""
                kw_ps = kw_alloc(i)
                for d in range(DT):
                    nc.tensor.matmul(
                        kw_ps[:],
                        a_all[:, d * M + i * 128:d * M + (i + 1) * 128],
                        pw_all[:, d * E:(d + 1) * E],
                        start=(d == 0), stop=False,
                    )
                nc.tensor.matmul(
                    kw_ps[:], ones_row[:], pb_sb[:], start=False, stop=True
                )
                sq = prok.tile([128, E], F32, tag="sq_kw", name=f"sq{i}")
                nsq = prok.tile([128, 1], F32, tag="nsq_kw", name=f"nsq{i}")
                tk = prok.tile([128, 1], F32, tag="tk", name=f"tk{i}")
                if act_norm:
                    nc.scalar.activation(sq[:], kw_ps[:], AF.Square, accum_out=nsq[:])
                    sk = prok.tile([128, 1], F32, tag="sk", name=f"sk{i}")
                    nc.scalar.activation(sk[:], nsq[:], AF.Sqrt)
                    rk = prok.tile([128, 1], F32, tag="rk", name=f"rk{i}")
                    nc.vector.reciprocal(rk[:], sk[:])
                    nc.vector.tensor_mul(tk[:], rk[:], rk[:])
                    nc.vector.tensor_mul(tk[:], tk[:], nsq[:])
                    nc.vector.tensor_scalar(tk[:], tk[:], -16.0, 48.0, OP.mult, OP.add)
                    nc.vector.tensor_mul(tk[:], tk[:], rk[:])
                else:
                    nc.vector.tensor_mul(sq[:], kw_ps[:], kw_ps[:])
                    nc.vector.tensor_reduce(nsq[:], sq[:], mybir.AxisListType.X, OP.add)
                    t1 = prok.tile([128, 1], mybir.dt.int32, tag="t1", name=f"t1_{i}")
                    nc.vector.tensor_scalar(
                        t1[:], nsq[:].bitcast(mybir.dt.int32), 1, 0,
                        OP.logical_shift_right, OP.logical_shift_right,
                    )
                    y0 = prok.tile([128, 1], mybir.dt.int32, tag="y0", name=f"y0_{i}")
                    nc.vector.tensor_scalar(y0[:], t1[:], -1, MAGIC, OP.mult, OP.add)
                    hs = prok.tile([128, 1], F32, tag="hs", name=f"hs{i}")
                    nc.vector.tensor_scalar(hs[:], nsq[:], 0.5, 0.0, OP.mult, OP.add)
                    ya = y0[:].bitcast(F32)
                    aa = prok.tile([128, 1], F32, tag="aa", name=f"aa{i}")
                    cc = prok.tile([128, 1], F32, tag="cc", name=f"cc{i}")
                    y1 = prok.tile([128, 1], F32, tag="y1", name=f"y1_{i}")
                    nc.vector.tensor_mul(aa[:], ya, ya)
                    nc.vector.tensor_mul(aa[:], aa[:], hs[:])
                    nc.vector.tensor_scalar(cc[:], aa[:], -1.0, 1.5, OP.mult, OP.add)
                    nc.vector.tensor_mul(y1[:], ya, cc[:])
                    nc.vector.tensor_mul(aa[:], y1[:], y1[:])
                    nc.vector.tensor_mul(aa[:], aa[:], hs[:])
                    nc.vector.tensor_scalar(cc[:], aa[:], -32.0, 48.0, OP.mult, OP.add)
                    nc.vector.tensor_mul(tk[:], y1[:], cc[:])
                kwn = prok.tile([128, E], BF16, tag="kwn", name=f"kwn{i}")
                nc.vector.tensor_scalar_mul(kwn[:], kw_ps[:], tk[:])
                for j in range(EC):
                    tpv = tp_alloc(i, j)
                    nc.tensor.transpose(tpv, kwn[:, j * 128:(j + 1) * 128], id_sb[:])
                    c2 = (j % 2) * MC + (i % 4) * 128
                    kpiece = kwnT[j // 2][i // 4][:, c2:c2 + 128]
                    nc.vector.tensor_copy(kpiece, tpv)
                    # fp8 rounding residual (32-scale) for the J-correction
                    nc.vector.tensor_sub(
                        res8[j // 2][i // 4][:, c2:c2 + 128], tpv, kpiece
                    )

            # ---- upfront prologue: m-tiles 0..3 in a pipelined 4-bank scope
            with (
                tc.tile_pool(name="pro_ps", bufs=2, space="PSUM") as pro_ps,
                tc.tile_pool(name="pro_ps2", bufs=2, space="PSUM") as pro_ps2,
            ):
                def kw_up(i):
                    return pro_ps.tile([128, E], F32, tag="kw_ps", name=f"kwps{i}")

                def tp_up(i, j):
                    t = pro_ps2.tile([128, 128], BF16, tag="tp", name=f"tp{i}_{j}")
                    return t[:]

                for i in range(UPFRONT):
                    pro_tile(i, kw_up, tp_up, act_norm=True)

            # ---- main loop. m-tiles 4..15 stream in through one PSUM bank
            # during mc0/mc1; that bank becomes a third score buffer for
            # mc2/mc3 (the exp->GEMM1 bank-recycle latency costs ~15% of the
            # exp pace at depth 2).
            with (
                tc.tile_pool(name="acc_ps", bufs=4, space="PSUM") as acc_ps,
                tc.tile_pool(name="d_ps", bufs=1, space="PSUM") as d_ps,
                tc.tile_pool(name="pp", bufs=30) as pp,
                tc.tile_pool(name="ob", bufs=8) as ob,
            ):
                def run_mc(mc, sc_ps, defer_gemm2):
                    def gemm1(kk):
                        tiles = []
                        for t in range(2):
                            k = 2 * kk + t
                            sc = sc_ps.tile([128, MC], F32, tag="sc", name=f"sc{kk}_{t}")
                            for j in range(2):
                                nc.tensor.matmul(
                                    sc[:],
                                    et3[j][:, :, k * 128:(k + 1) * 128],
                                    kwnT3[j][mc],
                                    start=(j == 0), stop=(j == 1),
                                    perf_mode=PM.DoubleRow,
                                )
                            tiles.append(sc)
                        return tiles

                    m0 = mc * MC
                    dacc = d_ps.tile([32, MC], F32, tag="dacc")

                    def gemm2_group(kk, p3, kwacc):
                        if kk == 0:
                            # J-correction opens each kwacc group (residuals
                            # are ready before the mc starts; PSUM addition is
                            # order-free), so the mc tail is just the last
                            # GEMM2 + the output copies
                            for j in range(EC):
                                for jj in range(2):
                                    nc.tensor.matmul(
                                        kwacc[j][:],
                                        k3[jj][:, :, j * 128:(j + 1) * 128],
                                        res8_3[jj][mc],
                                        start=(jj == 0), stop=False,
                                        perf_mode=PM.DoubleRow,
                                    )
                        en_kk = _pair(en_sb[:, kk * 2 * E:(kk + 1) * 2 * E], E)
                        for j in range(EC):
                            nc.tensor.matmul(
                                kwacc[j][:],
                                en_kk[:, :, j * 128:(j + 1) * 128],
                                p3,
                                start=False, stop=(kk == KK - 1),
                                perf_mode=PM.DoubleRow,
                            )

                    kwacc = None
                    if not defer_gemm2:
                        kwacc = [
                            acc_ps.tile([128, MC], F32, tag="kwacc", name=f"kwacc{j}")
                            for j in range(EC)
                        ]
                    saved = []
                    cur = gemm1(0)
                    for kk in range(KK):
                        p2 = pp.tile([128, 2 * MC], F8, tag="p2")
                        p3 = _pair(p2[:], MC)
                        for t in range(2):
                            nc.scalar.activation(
                                p2[:, t * MC:(t + 1) * MC], cur[t][:], AF.Exp,
                                scale=EXP_SCALE,
                            )
                        # issue next iteration's score GEMMs ahead of GEMM2 so
                        # the in-order PE queue never stalls behind the exps
                        if kk + 1 < KK:
                            cur = gemm1(kk + 1)
                        if defer_gemm2:
                            saved.append(p3)
                        else:
                            gemm2_group(kk, p3, kwacc)
                        sel = 32 if kk == KK - 1 else 0
                        nc.tensor.matmul(
                            dacc[:],
                            ones3[:, :, sel:sel + 32],
                            p3,
                            start=(kk == 0), stop=(kk == KK - 1),
                            perf_mode=PM.DoubleRow,
                        )
                    dsb = ob.tile([1, MC], F32, tag="dsb")
                    nc.vector.tensor_copy(dsb[:], dacc[0:1, :])
                    nc.sync.dma_start(out_d[:, m0:m0 + MC], dsb[:])
                    if defer_gemm2:
                        # mc0's weighted-sum sweep runs here, overlapped with
                        # the next mc, once the prologue's PSUM banks drained
                        kwacc = [
                            acc_ps.tile([128, MC], F32, tag="kwacc", name=f"kwacc{j}")
                            for j in range(EC)
                        ]
                        for kk in range(KK):
                            gemm2_group(kk, saved[kk], kwacc)
                    for j in range(EC):
                        osb = ob.tile([128, MC], F32, tag="osb")
                        nc.vector.tensor_copy(osb[:], kwacc[j][:])
                        nc.sync.dma_start(
                            out_pe[j * 128:(j + 1) * 128, m0:m0 + MC], osb[:]
                        )

                with tc.tile_pool(name="sc_all", bufs=3, space="PSUM") as sc_all:
                    for mc in range(NMC):
                        run_mc(mc, sc_all, defer_gemm2=(mc == 0))
    return nc


_CACHED = {}


def _get_program():
    if "nc" not in _CACHED:
        nc = build_program()
        _split_multiwait_ctrl(nc)
        _CACHED["nc"] = nc
    return _CACHED["nc"]


def _q8(x):
    return np.asarray(x, np.float32).astype(NPF8)


def _prep_in_maps(audio_feat, proj_w, proj_b, token_embedding):
    audio = np.asarray(audio_feat, np.float32).reshape(M, D)
    pw = np.asarray(proj_w, np.float32)
    pb = np.asarray(proj_b, np.float32).reshape(1, E)
    emb = np.asarray(token_embedding, np.float32)

    audio_t = np.ascontiguousarray(audio.T).astype(NPBF)
    pw_t = np.ascontiguousarray(pw).astype(NPBF)
    pb16 = pb.astype(NPBF)
    ident = np.eye(128, dtype=np.float32).astype(NPBF)
    # masked ones columns for the denominator matmul (dual-fp8 ldweights
    # needs >=32 stationary columns): [128, 2, 64] -> cols 0:32 all-real
    # pair, cols 32:64 last pair (tile 48 rows 0..31 real, tile 49 pad)
    nreal_last = VS - (VT - 2) * 128          # 32 real rows in tile 48
    onesv = np.zeros((128, 2, 64), np.float32)
    onesv[:, :, 0:32] = 1.0
    onesv[:nreal_last, 0, 32:64] = 1.0
    onesv = _q8(onesv.reshape(128, 128))

    in_maps = []
    for c in range(N_CORES):
        shard = np.zeros((VP, E), np.float32)
        shard[:VS] = emb[c * VS:(c + 1) * VS]
        nrm = np.maximum(np.linalg.norm(shard, axis=1, keepdims=True), 1e-8)
        emb_n = shard / nrm
        # et2: [jj, 128e, 2, VP] with component i = e-chunk (2*jj+i)
        et = (emb_n.T * N_SCALE).reshape(2, 2, 128, VP).transpose(0, 2, 1, 3)
        et2 = np.ascontiguousarray(_q8(et).reshape(2, 128, 2 * VP))
        # en2: [128v, KK, 2, E] with component i = v-tile (2*kk+i)
        en = (shard * W_SCALE).reshape(KK, 2, 128, E).transpose(2, 0, 1, 3)
        en2 = np.ascontiguousarray(_q8(en).reshape(128, KK * 2 * E))
        # correction stationary: KL[f, e] = q8(20 dbar S^T), S = shard^T emb_n
        S = shard.T @ emb_n                    # [e, f]
        kl = _q8(20.0 * DBAR_R * S.T)          # [f, e]
        k2 = np.ascontiguousarray(
            kl.reshape(2, 2, 128, E).transpose(0, 2, 1, 3).reshape(2, 128, 2 * E)
        )
        in_maps.append({
            "audio_t": audio_t,
            "pw_t": pw_t,
            "proj_b": pb16,
            "et2": et2,
            "en2": en2,
            "k2": k2,
            "onesv": onesv,
            "ident": ident,
        })
    return in_maps


def kernel(audio_feat, proj_w, proj_b, token_embedding, _trace=False):
    nc = _get_program()
    in_maps = _prep_in_maps(audio_feat, proj_w, proj_b, token_embedding)
    res = run_bass_kernel_spmd(
        nc, in_maps, core_ids=list(range(N_CORES)), trace=_trace
    )
    pe = np.zeros((E, M), np.float64)
    dn = np.zeros((1, M), np.float64)
    for c in range(N_CORES):
        pe += res.results[c]["out_pe"]
        dn += res.results[c]["out_d"]
    out = (pe / W_SCALE / dn).T.reshape(B, N, E).astype(np.float32)
    if _trace:
        return out, res
    return out


# revision 55
# speedup vs baseline: 1.0622x; 1.0334x over previous
"""VQ codebook kernel (nn_KW_CascadedBranch) for 8 Trainium2 NeuronCores.

Reference computation:
    kw   = audio_feat @ proj_w + proj_b                  [B,N,512]
    cos  = normalize(kw) @ normalize(token_embedding).T  [B,N,V]
    p    = softmax(cos / 0.1)
    out  = p @ token_embedding                           [B,N,512]

Strategy: tensor-parallel over the vocab dim V=49408. Each core owns a
6176-row shard (padded to 6400 = 50*128 = 25 DoubleRow pairs), keeps both
embedding layouts resident in SBUF as fp8e4, and computes the partial
(p @ emb) plus the partial softmax denominator for ALL B*N=2048 slots.
Softmax needs no max subtraction: logits = 10*cos are in [-10,10].
Host combines the 8 partials: out = (sum_c pe_c)/64 / (sum_c d_c).

The two big GEMMs (cos-scores, prob@emb) run on the PE in fp8e4 with
MatmulPerfMode.DoubleRow: two 128-deep contraction slices per instruction
at 0.5 cycles/row, i.e. 4x the fp32r rate. Quantization scales are powers
of two folded into the exp scale and the host epilogue:
  kwn*32, emb_n*32 -> scores_psum = 1024*cos, exp scale = 10/1024,
  emb*64           -> out_pe = 64*numerator.
The projection runs in bf16 (its quantization noise is amplified ~40x by
the flat-softmax cancellation, so fp8 is not enough there), with proj_b
folded in as a rank-1 row of the same PSUM accumulation group. Embedding
row norms are host-precomputed weight prep; vocab-pad rows are zeros so
exp(0)=1 there, and the denominator matmul uses a masked ones stationary
(>=32 columns, a dual-fp8 ldweights requirement) to exclude them exactly.

fp8 error feedback: out is a near-cancelling average over ~40k vocab rows
(|out|_rms ~ sigma_emb/200), so the kwn fp8 rounding error delta couples
through J = 10*Cov_p(emb, emb_n) into an output error ~40x larger than
naive estimates. Softmax here is nearly flat (den/VS ~= exp(T^-2/(2*512))
uniformly, +-0.7% over m), so J is well approximated with flat weights:
delta_num ~= dbar*10*S^T@delta, with S = sum_shard emb emb_n^T a host
constant and dbar the spec-derived density constant. The kernel captures
delta (the fp8 rounding residual, written straight to fp8 during the
transpose copy) and adds q8(20*dbar*S^T) @ q8(res32) into the same kwacc
PSUM accumulation group: 2 extra DoubleRow matmuls per (j, mc).
Validated in numpy + HW: maxrel 3.4e-2 -> 8.2e-3.

Scheduling: engine queues are in-order, so the 16-m-tile projection
prologue is split: tiles 0-3 run up front in a 4-bank pipelined scope
(closed before the main pools open), tiles 4-15 are emitted inside the
mc0/mc1 kk loops through a single shared PSUM bank, filling the PE's
slack under the ACT-bound exp stream. GEMM1 for iteration kk+1 issues
ahead of GEMM2(kk) so the PE queue never waits on the exps.
"""

import numpy as np
import ml_dtypes

import concourse.bass as bass
import concourse.mybir as mybir
from concourse import tile
from concourse.bass_utils import run_bass_kernel_spmd

F32 = mybir.dt.float32
F8 = mybir.dt.float8e4
BF16 = mybir.dt.bfloat16
AF = mybir.ActivationFunctionType
OP = mybir.AluOpType
PM = mybir.MatmulPerfMode
NPF8 = ml_dtypes.float8_e4m3
NPBF = ml_dtypes.bfloat16

N_CORES = 8
B, N, D, E, V = 256, 8, 768, 512, 49408
M = B * N                      # 2048 keyword slots
VS = V // N_CORES              # 6176 real vocab rows per core
VT = 50                        # v-tiles of 128 per core (6400 rows, 224 pad)
VP = VT * 128
KK = VT // 2                   # 25 DoubleRow v-tile pairs
MC = 512                       # m-chunk (columns per PSUM accumulator)
NMC = M // MC                  # 4
MT = M // 128                  # 16 m-tiles in the projection prologue
DT = D // 128                  # 6 d-chunks
DD = D // 256                  # 3 DoubleRow d-chunk pairs
EC = E // 128                  # 4 e-chunks (2 DoubleRow pairs)
EXP_SCALE = 10.0 / 1024.0      # 1/T divided by the 32*32 quant scales
W_SCALE = 64.0                 # emb quant scale
N_SCALE = 32.0                 # kwn / emb_n quant scale
DBAR_R = float(np.exp(100.0 / 1024.0))  # E[den]/VS for unit-norm randn data

# (mc, kk) -> prologue m-tile emitted at that point of the main loop
PRO_SCHED = {}
UPFRONT = 16


def _split_multiwait_ctrl(nc, max_waits: int = 1) -> int:
    """This container's walrus rejects instructions carrying more than one
    semaphore wait (CTRL and S3_LW encodings alike). Hoist overflow waits
    onto same-engine NoOps inserted immediately before the offender."""
    n_split = 0
    for fn in nc.m.functions:
        for bb in fn.blocks:
            rebuilt, changed = [], False
            for ins in bb.instructions:
                si = ins.sync_info
                if (
                    si is not None
                    and si.on_wait
                    and len(si.on_wait) > max_waits
                ):
                    waits = list(si.on_wait)
                    head, tail = waits[:-max_waits], waits[-max_waits:]
                    for i in range(0, len(head), max_waits):
                        nop = mybir.InstNoOp(name=f"{ins.name}-ws{i}", ins=[], outs=[])
                        nop.engine = ins.engine
                        nop.sync_info = mybir.SyncInfo(
                            on_wait=head[i:i + max_waits], on_update=[]
                        )
                        rebuilt.append(nop)
                    ins.sync_info = mybir.SyncInfo(
                        on_wait=tail, on_update=list(si.on_update or [])
                    )
                    changed = True
                    n_split += 1
                rebuilt.append(ins)
            if changed:
                bb.instructions = rebuilt
    return n_split


def _pair(ap2d, width):
    """View a flat [128, 2*width] AP as [128, 2, width]."""
    return ap2d.rearrange("p (a w) -> p a w", a=2, w=width)


def build_program():
    nc = bass.Bass(target_bir_lowering=False)

    a82 = nc.dram_tensor("a82", [2 * DD, 128, 2 * M], F8, kind="ExternalInput")
    pw82 = nc.dram_tensor("pw82", [2 * DD, 128, 2 * E], F8, kind="ExternalInput")
    proj_b = nc.dram_tensor("proj_b", [1, E], BF16, kind="ExternalInput")
    et2 = nc.dram_tensor("et2", [2, 128, 2 * VP], F8, kind="ExternalInput")
    en2 = nc.dram_tensor("en2", [128, KK * 2 * E], F8, kind="ExternalInput")
    k2 = nc.dram_tensor("k2", [2, 128, 2 * E], F8, kind="ExternalInput")
    onesv = nc.dram_tensor("onesv", [128, 128], F8, kind="ExternalInput")
    ident = nc.dram_tensor("ident", [128, 128], BF16, kind="ExternalInput")

    out_pe = nc.dram_tensor("out_pe", [E, M], F32, kind="ExternalOutput")
    out_d = nc.dram_tensor("out_d", [1, M], F32, kind="ExternalOutput")

    with tile.TileContext(nc) as tc:
        with (
            tc.tile_pool(name="resident", bufs=1) as res,
            tc.tile_pool(name="small", bufs=1) as small,
            tc.tile_pool(name="prok", bufs=6) as prok,
        ):
            # ---- resident SBUF tiles; DMAs ordered so compute starts early:
            # pw+audio chunks (prologue) -> et2 slices (GEMM1) -> en2 (GEMM2)
            pw8f = res.tile([128, 2 * DD * 2 * E], F8, tag="pw8f")
            pb_sb = small.tile([1, E], BF16, tag="pb")
            id_sb = small.tile([128, 128], BF16, tag="ident")
            a8f = res.tile([128, 2 * DD * 2 * M], F8, tag="a8f")
            et_all = res.tile([128, 4 * VP], F8, tag="et_all")
            ones_sb = small.tile([128, 128], F8, tag="ones_sb")
            en_sb = res.tile([128, KK * 2 * E], F8, tag="en")
            k_all = res.tile([128, 4 * E], F8, tag="k_all")

            et3 = [
                _pair(et_all[:, j * 2 * VP:(j + 1) * 2 * VP], VP) for j in range(2)
            ]
            etd = [_pair(et2[j], VP) for j in range(2)]
            k4 = k_all[:].rearrange("p (j x) -> p j x", j=2, x=2 * E)
            k4s = k2[:].rearrange("j p x -> p j x")
            # audio/pw hi-lo group views: g in 0..2 = hi d-pairs, 3..5 = lo
            av = [_pair(a8f[:, g * 2 * M:(g + 1) * 2 * M], M) for g in range(2 * DD)]
            avs = [_pair(a82[g], M) for g in range(2 * DD)]
            pwv = [_pair(pw8f[:, g * 2 * E:(g + 1) * 2 * E], E) for g in range(2 * DD)]
            # interleave the streams, fewest DMAs (each costs ~300ns of queue
            # overhead): audio m-chunk 0 + first et quarter feed the upfront
            # prologue and GEMM1(kk=0); the correction stationary and the
            # first en piece land before the kk=0 GEMM2 group opens; the
            # trailing et quarters and en pieces arrive mid-loop (mc0's
            # deferred GEMM2 sweep tolerates late en)
            QW = VP // 4
            ENP = (KK * 2 * E) // 5
            nc.sync.dma_start(
                pw8f[:].rearrange("p (g x) -> p g x", g=2 * DD, x=2 * E),
                pw82[:].rearrange("g p x -> p g x"),
            )
            nc.sync.dma_start(pb_sb[:], proj_b[:])
            nc.sync.dma_start(id_sb[:], ident[:])

            def a_dma(lo, hi):
                for g in range(2 * DD):
                    nc.sync.dma_start(
                        av[g][:, :, lo * 128:hi * 128], avs[g][:, :, lo * 128:hi * 128]
                    )

            a_dma(0, 4)
            for j in range(2):
                nc.sync.dma_start(et3[j][:, :, 0:QW], etd[j][:, :, 0:QW])
            nc.sync.dma_start(ones_sb[:], onesv[:])
            nc.sync.dma_start(k4[:], k4s[:])
            nc.sync.dma_start(en_sb[:, 0:ENP], en2[:, 0:ENP])
            a_dma(4, 8)
            for j in range(2):
                nc.sync.dma_start(et3[j][:, :, QW:2 * QW], etd[j][:, :, QW:2 * QW])
            a_dma(8, 12)
            nc.sync.dma_start(en_sb[:, ENP:2 * ENP], en2[:, ENP:2 * ENP])
            for j in range(2):
                nc.sync.dma_start(et3[j][:, :, 2 * QW:3 * QW], etd[j][:, :, 2 * QW:3 * QW])
            a_dma(12, 16)
            for j in range(2):
                nc.sync.dma_start(et3[j][:, :, 3 * QW:4 * QW], etd[j][:, :, 3 * QW:4 * QW])
            for pc in range(2, 5):
                sl = slice(pc * ENP, (pc + 1) * ENP)
                nc.sync.dma_start(en_sb[:, sl], en2[:, sl])
            kwnT = [
                [
                    res.tile([128, 2 * MC], F8, tag=f"kwnT{j}_{c}", name=f"kwnT{j}_{c}")
                    for c in range(NMC)
                ]
                for j in range(2)
            ]
            res8 = [
                [
                    res.tile([128, 2 * MC], F8, tag=f"res8_{j}_{c}", name=f"res8_{j}_{c}")
                    for c in range(NMC)
                ]
                for j in range(2)
            ]
            ones_row = small.tile([1, 128], BF16, tag="ones_row")
            nc.vector.memset(ones_row[:], 1.0)

            kwnT3 = [[_pair(kwnT[j][c][:], MC) for c in range(NMC)] for j in range(2)]
            res8_3 = [[_pair(res8[j][c][:], MC) for c in range(NMC)] for j in range(2)]
            k3 = [_pair(k_all[:, j * 2 * E:(j + 1) * 2 * E], E) for j in range(2)]
            ones3 = _pair(ones_sb[:], 64)

            MAGIC = 0x5F3759DF

            def pro_tile(i, kw_alloc, tp_alloc, act_norm):
                """Projection + normalize + transpose + fp8/residual capture
                for m-tile i. kw_alloc/tp_alloc hand out PSUM tiles. The
                upfront tiles use ACT Square+Sqrt (ACT is idle at start); the
                in-loop tiles keep ACT exp-only and compute 32*rsqrt on DVE
                via the 0x5f3759df bit trick + two Newton steps."""
                kw_ps = kw_alloc(i)
                # 3-pass fp8 DoubleRow projection: hi.hi + lo.hi + hi.lo with
                # residuals stored at the SAME scale (fp8 subnormals carry
                # them), so all passes accumulate into one PSUM group
                msl = slice(i * 128, (i + 1) * 128)
                passes = [(dd, dd) for dd in range(DD)] + \
                         [(DD + dd, dd) for dd in range(DD)] + \
                         [(dd, DD + dd) for dd in range(DD)]
                for n, (ga, gw) in enumerate(passes):
                    nc.tensor.matmul(
                        kw_ps[:],
                        av[ga][:, :, msl],
                        pwv[gw],
                        start=(n == 0), stop=False,
                        perf_mode=PM.DoubleRow,
                    )
                nc.tensor.matmul(
                    kw_ps[:], ones_row[:], pb_sb[:], start=False, stop=True
                )
                sq = prok.tile([128, E], F32, tag="sq_kw", name=f"sq{i}")
                nsq = prok.tile([128, 1], F32, tag="nsq_kw", name=f"nsq{i}")
                tk = prok.tile([128, 1], F32, tag="tk", name=f"tk{i}")
                if act_norm:
                    nc.scalar.activation(sq[:], kw_ps[:], AF.Square, accum_out=nsq[:])
                    sk = prok.tile([128, 1], F32, tag="sk", name=f"sk{i}")
                    nc.scalar.activation(sk[:], nsq[:], AF.Sqrt)
                    rk = prok.tile([128, 1], F32, tag="rk", name=f"rk{i}")
                    nc.vector.reciprocal(rk[:], sk[:])
                    nc.vector.tensor_mul(tk[:], rk[:], rk[:])
                    nc.vector.tensor_mul(tk[:], tk[:], nsq[:])
                    nc.vector.tensor_scalar(tk[:], tk[:], -16.0, 48.0, OP.mult, OP.add)
                    nc.vector.tensor_mul(tk[:], tk[:], rk[:])
                else:
                    nc.vector.tensor_mul(sq[:], kw_ps[:], kw_ps[:])
                    nc.vector.tensor_reduce(nsq[:], sq[:], mybir.AxisListType.X, OP.add)
                    t1 = prok.tile([128, 1], mybir.dt.int32, tag="t1", name=f"t1_{i}")
                    nc.vector.tensor_scalar(
                        t1[:], nsq[:].bitcast(mybir.dt.int32), 1, 0,
                        OP.logical_shift_right, OP.logical_shift_right,
                    )
                    y0 = prok.tile([128, 1], mybir.dt.int32, tag="y0", name=f"y0_{i}")
                    nc.vector.tensor_scalar(y0[:], t1[:], -1, MAGIC, OP.mult, OP.add)
                    hs = prok.tile([128, 1], F32, tag="hs", name=f"hs{i}")
                    nc.vector.tensor_scalar(hs[:], nsq[:], 0.5, 0.0, OP.mult, OP.add)
                    ya = y0[:].bitcast(F32)
                    aa = prok.tile([128, 1], F32, tag="aa", name=f"aa{i}")
                    cc = prok.tile([128, 1], F32, tag="cc", name=f"cc{i}")
                    y1 = prok.tile([128, 1], F32, tag="y1", name=f"y1_{i}")
                    nc.vector.tensor_mul(aa[:], ya, ya)
                    nc.vector.tensor_mul(aa[:], aa[:], hs[:])
                    nc.vector.tensor_scalar(cc[:], aa[:], -1.0, 1.5, OP.mult, OP.add)
                    nc.vector.tensor_mul(y1[:], ya, cc[:])
                    nc.vector.tensor_mul(aa[:], y1[:], y1[:])
                    nc.vector.tensor_mul(aa[:], aa[:], hs[:])
                    nc.vector.tensor_scalar(cc[:], aa[:], -32.0, 48.0, OP.mult, OP.add)
                    nc.vector.tensor_mul(tk[:], y1[:], cc[:])
                kwn = prok.tile([128, E], BF16, tag="kwn", name=f"kwn{i}")
                nc.vector.tensor_scalar_mul(kwn[:], kw_ps[:], tk[:])
                for j in range(EC):
                    tpv = tp_alloc(i, j)
                    nc.tensor.transpose(tpv, kwn[:, j * 128:(j + 1) * 128], id_sb[:])
                    c2 = (j % 2) * MC + (i % 4) * 128
                    kpiece = kwnT[j // 2][i // 4][:, c2:c2 + 128]
                    nc.vector.tensor_copy(kpiece, tpv)
                    # fp8 rounding residual (32-scale) for the J-correction
                    nc.vector.tensor_sub(
                        res8[j // 2][i // 4][:, c2:c2 + 128], tpv, kpiece
                    )

            # ---- upfront prologue: m-tiles 0..3 in a pipelined 4-bank scope
            with (
                tc.tile_pool(name="pro_ps", bufs=2, space="PSUM") as pro_ps,
                tc.tile_pool(name="pro_ps2", bufs=2, space="PSUM") as pro_ps2,
            ):
                def kw_up(i):
                    return pro_ps.tile([128, E], F32, tag="kw_ps", name=f"kwps{i}")

                def tp_up(i, j):
                    t = pro_ps2.tile([128, 128], BF16, tag="tp", name=f"tp{i}_{j}")
                    return t[:]

                for i in range(UPFRONT):
                    pro_tile(i, kw_up, tp_up, act_norm=True)

            # ---- main loop. m-tiles 4..15 stream in through one PSUM bank
            # during mc0/mc1; that bank becomes a third score buffer for
            # mc2/mc3 (the exp->GEMM1 bank-recycle latency costs ~15% of the
            # exp pace at depth 2).
            with (
                tc.tile_pool(name="acc_ps", bufs=4, space="PSUM") as acc_ps,
                tc.tile_pool(name="d_ps", bufs=1, space="PSUM") as d_ps,
                tc.tile_pool(name="pp", bufs=30) as pp,
                tc.tile_pool(name="ob", bufs=8) as ob,
            ):
                def run_mc(mc, sc_ps, defer_gemm2):
                    def gemm1(kk):
                        tiles = []
                        for t in range(2):
                            k = 2 * kk + t
                            sc = sc_ps.tile([128, MC], F32, tag="sc", name=f"sc{kk}_{t}")
                            for j in range(2):
                                nc.tensor.matmul(
                                    sc[:],
                                    et3[j][:, :, k * 128:(k + 1) * 128],
                                    kwnT3[j][mc],
                                    start=(j == 0), stop=(j == 1),
                                    perf_mode=PM.DoubleRow,
                                )
                            tiles.append(sc)
                        return tiles

                    m0 = mc * MC
                    dacc = d_ps.tile([32, MC], F32, tag="dacc")

                    def gemm2_group(kk, p3, kwacc):
                        if kk == 0:
                            # J-correction opens each kwacc group (residuals
                            # are ready before the mc starts; PSUM addition is
                            # order-free), so the mc tail is just the last
                            # GEMM2 + the output copies
                            for j in range(EC):
                                for jj in range(2):
                                    nc.tensor.matmul(
                                        kwacc[j][:],
                                        k3[jj][:, :, j * 128:(j + 1) * 128],
                                        res8_3[jj][mc],
                                        start=(jj == 0), stop=False,
                                        perf_mode=PM.DoubleRow,
                                    )
                        en_kk = _pair(en_sb[:, kk * 2 * E:(kk + 1) * 2 * E], E)
                        for j in range(EC):
                            nc.tensor.matmul(
                                kwacc[j][:],
                                en_kk[:, :, j * 128:(j + 1) * 128],
                                p3,
                                start=False, stop=(kk == KK - 1),
                                perf_mode=PM.DoubleRow,
                            )

                    kwacc = None
                    if not defer_gemm2:
                        kwacc = [
                            acc_ps.tile([128, MC], F32, tag="kwacc", name=f"kwacc{j}")
                            for j in range(EC)
                        ]
                    saved = []
                    cur = gemm1(0)
                    for kk in range(KK):
                        p2 = pp.tile([128, 2 * MC], F8, tag="p2")
                        p3 = _pair(p2[:], MC)
                        for t in range(2):
                            nc.scalar.activation(
                                p2[:, t * MC:(t + 1) * MC], cur[t][:], AF.Exp,
                                scale=EXP_SCALE,
                            )
                        # issue next iteration's score GEMMs ahead of GEMM2 so
                        # the in-order PE queue never stalls behind the exps
                        if kk + 1 < KK:
                            cur = gemm1(kk + 1)
                        if defer_gemm2:
                            saved.append(p3)
                        else:
                            gemm2_group(kk, p3, kwacc)
                        sel = 32 if kk == KK - 1 else 0
                        nc.tensor.matmul(
                            dacc[:],
                            ones3[:, :, sel:sel + 32],
                            p3,
                            start=(kk == 0), stop=(kk == KK - 1),
                            perf_mode=PM.DoubleRow,
                        )
                    dsb = ob.tile([1, MC], F32, tag="dsb")
                    nc.vector.tensor_copy(dsb[:], dacc[0:1, :])
                    nc.sync.dma_start(out_d[:, m0:m0 + MC], dsb[:])
                    if defer_gemm2:
                        # mc0's weighted-sum sweep runs here, overlapped with
                        # the next mc, once the prologue's PSUM banks drained
                        kwacc = [
                            acc_ps.tile([128, MC], F32, tag="kwacc", name=f"kwacc{j}")
                            for j in range(EC)
                        ]
                        for kk in range(KK):
                            gemm2_group(kk, saved[kk], kwacc)
                    for j in range(EC):
                        osb = ob.tile([128, MC], F32, tag="osb")
                        nc.vector.tensor_copy(osb[:], kwacc[j][:])
                        nc.sync.dma_start(
                            out_pe[j * 128:(j + 1) * 128, m0:m0 + MC], osb[:]
                        )

                with tc.tile_pool(name="sc_all", bufs=3, space="PSUM") as sc_all:
                    for mc in range(NMC):
                        run_mc(mc, sc_all, defer_gemm2=(mc == 0))
    return nc


_CACHED = {}


def _get_program():
    if "nc" not in _CACHED:
        nc = build_program()
        _split_multiwait_ctrl(nc)
        _CACHED["nc"] = nc
    return _CACHED["nc"]


def _q8(x):
    return np.asarray(x, np.float32).astype(NPF8)


def _prep_in_maps(audio_feat, proj_w, proj_b, token_embedding):
    audio = np.asarray(audio_feat, np.float32).reshape(M, D)
    pw = np.asarray(proj_w, np.float32)
    pb = np.asarray(proj_b, np.float32).reshape(1, E)
    emb = np.asarray(token_embedding, np.float32)

    def _hilo_pairs(x, scale):
        """[D, X] -> fp8 hi/lo stacked [2*DD, 128, 2*X] in DoubleRow d-pair
        layout; lo = same-scale residual (lives in fp8 subnormal range)."""
        xs = x * scale
        hi = _q8(xs)
        lo = _q8(xs - hi.astype(np.float32))
        out = []
        for h in (hi, lo):
            out.append(h.reshape(DD, 2, 128, -1).transpose(0, 2, 1, 3))
        return np.ascontiguousarray(
            np.concatenate(out, 0).reshape(2 * DD, 128, -1)
        )

    a82 = _hilo_pairs(audio.T, 1.0)
    pw82 = _hilo_pairs(pw, W_SCALE)
    pb16 = (pb * W_SCALE).astype(NPBF)
    ident = np.eye(128, dtype=np.float32).astype(NPBF)
    # masked ones columns for the denominator matmul (dual-fp8 ldweights
    # needs >=32 stationary columns): [128, 2, 64] -> cols 0:32 all-real
    # pair, cols 32:64 last pair (tile 48 rows 0..31 real, tile 49 pad)
    nreal_last = VS - (VT - 2) * 128          # 32 real rows in tile 48
    onesv = np.zeros((128, 2, 64), np.float32)
    onesv[:, :, 0:32] = 1.0
    onesv[:nreal_last, 0, 32:64] = 1.0
    onesv = _q8(onesv.reshape(128, 128))

    in_maps = []
    for c in range(N_CORES):
        shard = np.zeros((VP, E), np.float32)
        shard[:VS] = emb[c * VS:(c + 1) * VS]
        nrm = np.maximum(np.linalg.norm(shard, axis=1, keepdims=True), 1e-8)
        emb_n = shard / nrm
        # et2: [jj, 128e, 2, VP] with component i = e-chunk (2*jj+i)
        et = (emb_n.T * N_SCALE).reshape(2, 2, 128, VP).transpose(0, 2, 1, 3)
        et2 = np.ascontiguousarray(_q8(et).reshape(2, 128, 2 * VP))
        # en2: [128v, KK, 2, E] with component i = v-tile (2*kk+i)
        en = (shard * W_SCALE).reshape(KK, 2, 128, E).transpose(2, 0, 1, 3)
        en2 = np.ascontiguousarray(_q8(en).reshape(128, KK * 2 * E))
        # correction stationary: KL[f, e] = q8(20 dbar S^T), S = shard^T emb_n
        S = shard.T @ emb_n                    # [e, f]
        kl = _q8(20.0 * DBAR_R * S.T)          # [f, e]
        k2 = np.ascontiguousarray(
            kl.reshape(2, 2, 128, E).transpose(0, 2, 1, 3).reshape(2, 128, 2 * E)
        )
        in_maps.append({
            "a82": a82,
            "pw82": pw82,
            "proj_b": pb16,
            "et2": et2,
            "en2": en2,
            "k2": k2,
            "onesv": onesv,
            "ident": ident,
        })
    return in_maps


def kernel(audio_feat, proj_w, proj_b, token_embedding, _trace=False):
    nc = _get_program()
    in_maps = _prep_in_maps(audio_feat, proj_w, proj_b, token_embedding)
    res = run_bass_kernel_spmd(
        nc, in_maps, core_ids=list(range(N_CORES)), trace=_trace
    )
    pe = np.zeros((E, M), np.float64)
    dn = np.zeros((1, M), np.float64)
    for c in range(N_CORES):
        pe += res.results[c]["out_pe"]
        dn += res.results[c]["out_d"]
    out = (pe / W_SCALE / dn).T.reshape(B, N, E).astype(np.float32)
    if _trace:
        return out, res
    return out


# revision 65
# speedup vs baseline: 1.1268x; 1.0608x over previous
"""VQ codebook kernel (nn_KW_CascadedBranch) for 8 Trainium2 NeuronCores.

Reference computation:
    kw   = audio_feat @ proj_w + proj_b                  [B,N,512]
    cos  = normalize(kw) @ normalize(token_embedding).T  [B,N,V]
    p    = softmax(cos / 0.1)
    out  = p @ token_embedding                           [B,N,512]

Strategy: tensor-parallel over the vocab dim V=49408. Each core owns a
6176-row shard (padded to 6400 = 50*128 = 25 DoubleRow pairs), keeps both
embedding layouts resident in SBUF as fp8e4, and computes the partial
(p @ emb) plus the partial softmax denominator for ALL B*N=2048 slots.
Softmax needs no max subtraction: logits = 10*cos are in [-10,10].
Host combines the 8 partials: out = (sum_c pe_c)/64 / (sum_c d_c).

The two big GEMMs (cos-scores, prob@emb) run on the PE in fp8e4 with
MatmulPerfMode.DoubleRow: two 128-deep contraction slices per instruction
at 0.5 cycles/row, i.e. 4x the fp32r rate. Quantization scales are powers
of two folded into the exp scale and the host epilogue:
  kwn*32, emb_n*32 -> scores_psum = 1024*cos, exp scale = 10/1024,
  emb*64           -> out_pe = 64*numerator.
The projection runs in bf16 (its quantization noise is amplified ~40x by
the flat-softmax cancellation, so fp8 is not enough there), with proj_b
folded in as a rank-1 row of the same PSUM accumulation group. Embedding
row norms are host-precomputed weight prep; vocab-pad rows are zeros so
exp(0)=1 there, and the denominator matmul uses a masked ones stationary
(>=32 columns, a dual-fp8 ldweights requirement) to exclude them exactly.

fp8 error feedback: out is a near-cancelling average over ~40k vocab rows
(|out|_rms ~ sigma_emb/200), so the kwn fp8 rounding error delta couples
through J = 10*Cov_p(emb, emb_n) into an output error ~40x larger than
naive estimates. Softmax here is nearly flat (den/VS ~= exp(T^-2/(2*512))
uniformly, +-0.7% over m), so J is well approximated with flat weights:
delta_num ~= dbar*10*S^T@delta, with S = sum_shard emb emb_n^T a host
constant and dbar the spec-derived density constant. The kernel captures
delta (the fp8 rounding residual, written straight to fp8 during the
transpose copy) and adds q8(20*dbar*S^T) @ q8(res32) into the same kwacc
PSUM accumulation group: 2 extra DoubleRow matmuls per (j, mc).
Validated in numpy + HW: maxrel 3.4e-2 -> 8.2e-3.

Scheduling: engine queues are in-order, so the 16-m-tile projection
prologue is split: tiles 0-3 run up front in a 4-bank pipelined scope
(closed before the main pools open), tiles 4-15 are emitted inside the
mc0/mc1 kk loops through a single shared PSUM bank, filling the PE's
slack under the ACT-bound exp stream. GEMM1 for iteration kk+1 issues
ahead of GEMM2(kk) so the PE queue never waits on the exps.
"""

import numpy as np
import ml_dtypes

import concourse.bass as bass
import concourse.mybir as mybir
from concourse import tile
from concourse.bass_utils import run_bass_kernel_spmd

F32 = mybir.dt.float32
F8 = mybir.dt.float8e4
BF16 = mybir.dt.bfloat16
AF = mybir.ActivationFunctionType
OP = mybir.AluOpType
PM = mybir.MatmulPerfMode
NPF8 = ml_dtypes.float8_e4m3
NPBF = ml_dtypes.bfloat16

N_CORES = 8
B, N, D, E, V = 256, 8, 768, 512, 49408
M = B * N                      # 2048 keyword slots
VS = V // N_CORES              # 6176 real vocab rows per core
VT = 50                        # v-tiles of 128 per core (6400 rows, 224 pad)
VP = VT * 128
KK = VT // 2                   # 25 DoubleRow v-tile pairs
MC = 512                       # m-chunk (columns per PSUM accumulator)
NMC = M // MC                  # 4
MT = M // 128                  # 16 m-tiles in the projection prologue
DT = D // 128                  # 6 d-chunks
DD = D // 256                  # 3 DoubleRow d-chunk pairs
EC = E // 128                  # 4 e-chunks (2 DoubleRow pairs)
EXP_SCALE = 10.0 / 1024.0      # 1/T divided by the 32*32 quant scales
W_SCALE = 64.0                 # emb quant scale
N_SCALE = 32.0                 # kwn / emb_n quant scale
DBAR_R = float(np.exp(100.0 / 1024.0))  # E[den]/VS for unit-norm randn data

# (mc, kk) -> prologue m-tile emitted at that point of the main loop
PRO_SCHED = {}
UPFRONT = 16


def _split_multiwait_ctrl(nc, max_waits: int = 1) -> int:
    """This container's walrus rejects instructions carrying more than one
    semaphore wait (CTRL and S3_LW encodings alike). Hoist overflow waits
    onto same-engine NoOps inserted immediately before the offender."""
    n_split = 0
    for fn in nc.m.functions:
        for bb in fn.blocks:
            rebuilt, changed = [], False
            for ins in bb.instructions:
                si = ins.sync_info
                if (
                    si is not None
                    and si.on_wait
                    and len(si.on_wait) > max_waits
                ):
                    waits = list(si.on_wait)
                    head, tail = waits[:-max_waits], waits[-max_waits:]
                    for i in range(0, len(head), max_waits):
                        nop = mybir.InstNoOp(name=f"{ins.name}-ws{i}", ins=[], outs=[])
                        nop.engine = ins.engine
                        nop.sync_info = mybir.SyncInfo(
                            on_wait=head[i:i + max_waits], on_update=[]
                        )
                        rebuilt.append(nop)
                    ins.sync_info = mybir.SyncInfo(
                        on_wait=tail, on_update=list(si.on_update or [])
                    )
                    changed = True
                    n_split += 1
                rebuilt.append(ins)
            if changed:
                bb.instructions = rebuilt
    return n_split


def _pair(ap2d, width):
    """View a flat [128, 2*width] AP as [128, 2, width]."""
    return ap2d.rearrange("p (a w) -> p a w", a=2, w=width)


def build_program():
    nc = bass.Bass(target_bir_lowering=False)

    a82 = nc.dram_tensor("a82", [2 * DD, 128, 2 * M], F8, kind="ExternalInput")
    pw82 = nc.dram_tensor("pw82", [2 * DD, 128, 2 * E], F8, kind="ExternalInput")
    proj_b = nc.dram_tensor("proj_b", [1, E], BF16, kind="ExternalInput")
    et2 = nc.dram_tensor("et2", [2, 128, 2 * VP], F8, kind="ExternalInput")
    en2 = nc.dram_tensor("en2", [128, KK * 2 * E], F8, kind="ExternalInput")
    k2 = nc.dram_tensor("k2", [2, 128, 2 * E], F8, kind="ExternalInput")
    onesv = nc.dram_tensor("onesv", [128, 128], F8, kind="ExternalInput")
    ident = nc.dram_tensor("ident", [128, 128], BF16, kind="ExternalInput")

    out_pe = nc.dram_tensor("out_pe", [E, M], F32, kind="ExternalOutput")
    out_d = nc.dram_tensor("out_d", [1, M], F32, kind="ExternalOutput")

    with tile.TileContext(nc) as tc:
        with (
            tc.tile_pool(name="resident", bufs=1) as res,
            tc.tile_pool(name="small", bufs=1) as small,
            tc.tile_pool(name="prok", bufs=4) as prok,
        ):
            # ---- resident SBUF tiles; DMAs ordered so compute starts early:
            # pw+audio chunks (prologue) -> et2 slices (GEMM1) -> en2 (GEMM2)
            pw8f = res.tile([128, 2 * DD * 2 * E], F8, tag="pw8f")
            pb_sb = small.tile([1, E], BF16, tag="pb")
            id_sb = small.tile([128, 128], BF16, tag="ident")
            a8f = res.tile([128, 2 * DD * 2 * M], F8, tag="a8f")
            et_all = res.tile([128, 4 * VP], F8, tag="et_all")
            ones_sb = small.tile([128, 128], F8, tag="ones_sb")
            en_sb = res.tile([128, KK * 2 * E], F8, tag="en")
            k_all = res.tile([128, 4 * E], F8, tag="k_all")

            et3 = [
                _pair(et_all[:, j * 2 * VP:(j + 1) * 2 * VP], VP) for j in range(2)
            ]
            etd = [_pair(et2[j], VP) for j in range(2)]
            k4 = k_all[:].rearrange("p (j x) -> p j x", j=2, x=2 * E)
            k4s = k2[:].rearrange("j p x -> p j x")
            # audio/pw hi-lo group views: g in 0..2 = hi d-pairs, 3..5 = lo
            av = [_pair(a8f[:, g * 2 * M:(g + 1) * 2 * M], M) for g in range(2 * DD)]
            avs = [_pair(a82[g], M) for g in range(2 * DD)]
            pwv = [_pair(pw8f[:, g * 2 * E:(g + 1) * 2 * E], E) for g in range(2 * DD)]
            # interleave the streams, fewest DMAs (each costs ~300ns of queue
            # overhead): audio m-chunk 0 + first et quarter feed the upfront
            # prologue and GEMM1(kk=0); the correction stationary and the
            # first en piece land before the kk=0 GEMM2 group opens; the
            # trailing et quarters and en pieces arrive mid-loop (mc0's
            # deferred GEMM2 sweep tolerates late en)
            QW = VP // 4
            ENP = (KK * 2 * E) // 5
            nc.sync.dma_start(
                pw8f[:].rearrange("p (g x) -> p g x", g=2 * DD, x=2 * E),
                pw82[:].rearrange("g p x -> p g x"),
            )
            nc.sync.dma_start(pb_sb[:], proj_b[:])
            nc.sync.dma_start(id_sb[:], ident[:])

            def a_dma(lo, hi):
                for g in range(2 * DD):
                    nc.sync.dma_start(
                        av[g][:, :, lo * 128:hi * 128], avs[g][:, :, lo * 128:hi * 128]
                    )

            a_dma(0, 4)
            for j in range(2):
                nc.sync.dma_start(et3[j][:, :, 0:QW], etd[j][:, :, 0:QW])
            nc.sync.dma_start(ones_sb[:], onesv[:])
            nc.sync.dma_start(k4[:], k4s[:])
            nc.sync.dma_start(en_sb[:, 0:ENP], en2[:, 0:ENP])
            a_dma(4, 8)
            for j in range(2):
                nc.sync.dma_start(et3[j][:, :, QW:2 * QW], etd[j][:, :, QW:2 * QW])
            a_dma(8, 12)
            nc.sync.dma_start(en_sb[:, ENP:2 * ENP], en2[:, ENP:2 * ENP])
            for j in range(2):
                nc.sync.dma_start(et3[j][:, :, 2 * QW:3 * QW], etd[j][:, :, 2 * QW:3 * QW])
            a_dma(12, 16)
            for j in range(2):
                nc.sync.dma_start(et3[j][:, :, 3 * QW:4 * QW], etd[j][:, :, 3 * QW:4 * QW])
            for pc in range(2, 5):
                sl = slice(pc * ENP, (pc + 1) * ENP)
                nc.sync.dma_start(en_sb[:, sl], en2[:, sl])
            kwnT = [
                [
                    res.tile([128, 2 * MC], F8, tag=f"kwnT{j}_{c}", name=f"kwnT{j}_{c}")
                    for c in range(NMC)
                ]
                for j in range(2)
            ]
            res8 = [
                [
                    res.tile([128, 2 * MC], F8, tag=f"res8_{j}_{c}", name=f"res8_{j}_{c}")
                    for c in range(NMC)
                ]
                for j in range(2)
            ]
            ones_row = small.tile([1, 128], BF16, tag="ones_row")
            nc.vector.memset(ones_row[:], 1.0)

            kwnT3 = [[_pair(kwnT[j][c][:], MC) for c in range(NMC)] for j in range(2)]
            res8_3 = [[_pair(res8[j][c][:], MC) for c in range(NMC)] for j in range(2)]
            k3 = [_pair(k_all[:, j * 2 * E:(j + 1) * 2 * E], E) for j in range(2)]
            ones3 = _pair(ones_sb[:], 64)

            MAGIC = 0x5F3759DF

            def pro_tile(i, kw_alloc, tp_alloc, act_norm):
                """Projection + normalize + transpose + fp8/residual capture
                for m-tile i. kw_alloc/tp_alloc hand out PSUM tiles. The
                upfront tiles use ACT Square+Sqrt (ACT is idle at start); the
                in-loop tiles keep ACT exp-only and compute 32*rsqrt on DVE
                via the 0x5f3759df bit trick + two Newton steps."""
                kw_ps = kw_alloc(i)
                # 3-pass fp8 DoubleRow projection: hi.hi + lo.hi + hi.lo with
                # residuals stored at the SAME scale (fp8 subnormals carry
                # them), so all passes accumulate into one PSUM group
                msl = slice(i * 128, (i + 1) * 128)
                passes = [(dd, dd) for dd in range(DD)] + \
                         [(DD + dd, dd) for dd in range(DD)] + \
                         [(dd, DD + dd) for dd in range(DD)]
                for n, (ga, gw) in enumerate(passes):
                    nc.tensor.matmul(
                        kw_ps[:],
                        av[ga][:, :, msl],
                        pwv[gw],
                        start=(n == 0), stop=False,
                        perf_mode=PM.DoubleRow,
                    )
                nc.tensor.matmul(
                    kw_ps[:], ones_row[:], pb_sb[:], start=False, stop=True
                )
                sq = prok.tile([128, E], F32, tag="sq_kw", name=f"sq{i}")
                nsq = prok.tile([128, 1], F32, tag="nsq_kw", name=f"nsq{i}")
                tk = prok.tile([128, 1], F32, tag="tk", name=f"tk{i}")
                kw_src = kw_ps
                if act_norm:
                    nc.scalar.activation(sq[:], kw_ps[:], AF.Square, accum_out=nsq[:])
                    sk = prok.tile([128, 1], F32, tag="sk", name=f"sk{i}")
                    nc.scalar.activation(sk[:], nsq[:], AF.Sqrt)
                    rk = prok.tile([128, 1], F32, tag="rk", name=f"rk{i}")
                    nc.vector.reciprocal(rk[:], sk[:])
                    nc.vector.tensor_mul(tk[:], rk[:], rk[:])
                    nc.vector.tensor_mul(tk[:], tk[:], nsq[:])
                    nc.vector.tensor_scalar(tk[:], tk[:], -16.0, 48.0, OP.mult, OP.add)
                    nc.vector.tensor_mul(tk[:], tk[:], rk[:])
                else:
                    # DVE can read only one non-scalar input from PSUM: stage
                    # kw in SBUF (also frees the prologue PSUM bank earlier)
                    kwc = prok.tile([128, E], F32, tag="kwc", name=f"kwc{i}")
                    nc.vector.tensor_copy(kwc[:], kw_ps[:])
                    nc.vector.tensor_mul(sq[:], kwc[:], kwc[:])
                    nc.vector.tensor_reduce(nsq[:], sq[:], mybir.AxisListType.X, OP.add)
                    t1 = prok.tile([128, 1], mybir.dt.int32, tag="t1", name=f"t1_{i}")
                    nc.vector.tensor_scalar(
                        t1[:], nsq[:].bitcast(mybir.dt.int32), 1, 0,
                        OP.logical_shift_right, OP.logical_shift_right,
                    )
                    y0 = prok.tile([128, 1], mybir.dt.int32, tag="y0", name=f"y0_{i}")
                    nc.vector.tensor_scalar(y0[:], t1[:], -1, MAGIC, OP.mult, OP.add)
                    hs = prok.tile([128, 1], F32, tag="hs", name=f"hs{i}")
                    nc.vector.tensor_scalar(hs[:], nsq[:], 0.5, 0.0, OP.mult, OP.add)
                    ya = y0[:].bitcast(F32)
                    aa = prok.tile([128, 1], F32, tag="aa", name=f"aa{i}")
                    cc = prok.tile([128, 1], F32, tag="cc", name=f"cc{i}")
                    y1 = prok.tile([128, 1], F32, tag="y1", name=f"y1_{i}")
                    nc.vector.tensor_mul(aa[:], ya, ya)
                    nc.vector.tensor_mul(aa[:], aa[:], hs[:])
                    nc.vector.tensor_scalar(cc[:], aa[:], -1.0, 1.5, OP.mult, OP.add)
                    nc.vector.tensor_mul(y1[:], ya, cc[:])
                    nc.vector.tensor_mul(aa[:], y1[:], y1[:])
                    nc.vector.tensor_mul(aa[:], aa[:], hs[:])
                    nc.vector.tensor_scalar(cc[:], aa[:], -32.0, 48.0, OP.mult, OP.add)
                    nc.vector.tensor_mul(tk[:], y1[:], cc[:])
                    kw_src = kwc
                kwn = prok.tile([128, E], BF16, tag="kwn", name=f"kwn{i}")
                nc.vector.tensor_scalar_mul(kwn[:], kw_src[:], tk[:])
                for j in range(EC):
                    tpv = tp_alloc(i, j)
                    nc.tensor.transpose(tpv, kwn[:, j * 128:(j + 1) * 128], id_sb[:])
                    c2 = (j % 2) * MC + (i % 4) * 128
                    kpiece = kwnT[j // 2][i // 4][:, c2:c2 + 128]
                    nc.vector.tensor_copy(kpiece, tpv)
                    # fp8 rounding residual (32-scale) for the J-correction
                    nc.vector.tensor_sub(
                        res8[j // 2][i // 4][:, c2:c2 + 128], tpv, kpiece
                    )

            # ---- upfront prologue: m-tiles 0..3 in a pipelined 4-bank scope
            with (
                tc.tile_pool(name="pro_ps", bufs=2, space="PSUM") as pro_ps,
                tc.tile_pool(name="pro_ps2", bufs=2, space="PSUM") as pro_ps2,
            ):
                def kw_up(i):
                    return pro_ps.tile([128, E], F32, tag="kw_ps", name=f"kwps{i}")

                def tp_up(i, j):
                    t = pro_ps2.tile([128, 128], BF16, tag="tp", name=f"tp{i}_{j}")
                    return t[:]

                for i in range(UPFRONT):
                    pro_tile(i, kw_up, tp_up, act_norm=(i < 4))

            # ---- main loop. m-tiles 4..15 stream in through one PSUM bank
            # during mc0/mc1; that bank becomes a third score buffer for
            # mc2/mc3 (the exp->GEMM1 bank-recycle latency costs ~15% of the
            # exp pace at depth 2).
            with (
                tc.tile_pool(name="acc_ps", bufs=4, space="PSUM") as acc_ps,
                tc.tile_pool(name="d_ps", bufs=1, space="PSUM") as d_ps,
                tc.tile_pool(name="pp", bufs=58) as pp,
                tc.tile_pool(name="ob", bufs=5) as ob,
            ):
                def run_mc(mc, sc_ps, defer_gemm2):
                    def gemm1(kk):
                        tiles = []
                        for t in range(2):
                            k = 2 * kk + t
                            sc = sc_ps.tile([128, MC], F32, tag="sc", name=f"sc{kk}_{t}")
                            for j in range(2):
                                nc.tensor.matmul(
                                    sc[:],
                                    et3[j][:, :, k * 128:(k + 1) * 128],
                                    kwnT3[j][mc],
                                    start=(j == 0), stop=(j == 1),
                                    perf_mode=PM.DoubleRow,
                                )
                            tiles.append(sc)
                        return tiles

                    m0 = mc * MC
                    dacc = d_ps.tile([32, MC], F32, tag="dacc")

                    def gemm2_group(kk, p3, kwacc):
                        if kk == 0:
                            # J-correction opens each kwacc group (residuals
                            # are ready before the mc starts; PSUM addition is
                            # order-free), so the mc tail is just the last
                            # GEMM2 + the output copies
                            for j in range(EC):
                                for jj in range(2):
                                    nc.tensor.matmul(
                                        kwacc[j][:],
                                        k3[jj][:, :, j * 128:(j + 1) * 128],
                                        res8_3[jj][mc],
                                        start=(jj == 0), stop=False,
                                        perf_mode=PM.DoubleRow,
                                    )
                        en_kk = _pair(en_sb[:, kk * 2 * E:(kk + 1) * 2 * E], E)
                        for j in range(EC):
                            nc.tensor.matmul(
                                kwacc[j][:],
                                en_kk[:, :, j * 128:(j + 1) * 128],
                                p3,
                                start=False, stop=(kk == KK - 1),
                                perf_mode=PM.DoubleRow,
                            )

                    kwacc = None
                    if not defer_gemm2:
                        kwacc = [
                            acc_ps.tile([128, MC], F32, tag="kwacc", name=f"kwacc{j}")
                            for j in range(EC)
                        ]
                    saved = []
                    cur = gemm1(0)
                    for kk in range(KK):
                        p2 = pp.tile([128, 2 * MC], F8, tag="p2")
                        p3 = _pair(p2[:], MC)
                        for t in range(2):
                            nc.scalar.activation(
                                p2[:, t * MC:(t + 1) * MC], cur[t][:], AF.Exp,
                                scale=EXP_SCALE,
                            )
                        # issue next iteration's score GEMMs ahead of GEMM2 so
                        # the in-order PE queue never stalls behind the exps
                        if kk + 1 < KK:
                            cur = gemm1(kk + 1)
                        if defer_gemm2:
                            saved.append(p3)
                        else:
                            gemm2_group(kk, p3, kwacc)
                        sel = 32 if kk == KK - 1 else 0
                        nc.tensor.matmul(
                            dacc[:],
                            ones3[:, :, sel:sel + 32],
                            p3,
                            start=(kk == 0), stop=(kk == KK - 1),
                            perf_mode=PM.DoubleRow,
                        )
                    dsb = ob.tile([1, MC], F32, tag="dsb")
                    nc.vector.tensor_copy(dsb[:], dacc[0:1, :])
                    nc.sync.dma_start(out_d[:, m0:m0 + MC], dsb[:])
                    if defer_gemm2:
                        # mc0's weighted-sum sweep runs here, overlapped with
                        # the next mc, once the prologue's PSUM banks drained
                        kwacc = [
                            acc_ps.tile([128, MC], F32, tag="kwacc", name=f"kwacc{j}")
                            for j in range(EC)
                        ]
                        for kk in range(KK):
                            gemm2_group(kk, saved[kk], kwacc)
                    for j in range(EC):
                        osb = ob.tile([128, MC], F32, tag="osb")
                        nc.vector.tensor_copy(osb[:], kwacc[j][:])
                        nc.sync.dma_start(
                            out_pe[j * 128:(j + 1) * 128, m0:m0 + MC], osb[:]
                        )

                with tc.tile_pool(name="sc_all", bufs=3, space="PSUM") as sc_all:
                    for mc in range(NMC):
                        run_mc(mc, sc_all, defer_gemm2=(mc <= 1))
    return nc


_CACHED = {}


def _get_program():
    if "nc" not in _CACHED:
        nc = build_program()
        _split_multiwait_ctrl(nc)
        _CACHED["nc"] = nc
    return _CACHED["nc"]


def _q8(x):
    return np.asarray(x, np.float32).astype(NPF8)


def _prep_in_maps(audio_feat, proj_w, proj_b, token_embedding):
    audio = np.asarray(audio_feat, np.float32).reshape(M, D)
    pw = np.asarray(proj_w, np.float32)
    pb = np.asarray(proj_b, np.float32).reshape(1, E)
    emb = np.asarray(token_embedding, np.float32)

    def _hilo_pairs(x, scale):
        """[D, X] -> fp8 hi/lo stacked [2*DD, 128, 2*X] in DoubleRow d-pair
        layout; lo = same-scale residual (lives in fp8 subnormal range)."""
        xs = x * scale
        hi = _q8(xs)
        lo = _q8(xs - hi.astype(np.float32))
        out = []
        for h in (hi, lo):
            out.append(h.reshape(DD, 2, 128, -1).transpose(0, 2, 1, 3))
        return np.ascontiguousarray(
            np.concatenate(out, 0).reshape(2 * DD, 128, -1)
        )

    a82 = _hilo_pairs(audio.T, 1.0)
    pw82 = _hilo_pairs(pw, W_SCALE)
    pb16 = (pb * W_SCALE).astype(NPBF)
    ident = np.eye(128, dtype=np.float32).astype(NPBF)
    # masked ones columns for the denominator matmul (dual-fp8 ldweights
    # needs >=32 stationary columns): [128, 2, 64] -> cols 0:32 all-real
    # pair, cols 32:64 last pair (tile 48 rows 0..31 real, tile 49 pad)
    nreal_last = VS - (VT - 2) * 128          # 32 real rows in tile 48
    onesv = np.zeros((128, 2, 64), np.float32)
    onesv[:, :, 0:32] = 1.0
    onesv[:nreal_last, 0, 32:64] = 1.0
    onesv = _q8(onesv.reshape(128, 128))

    in_maps = []
    for c in range(N_CORES):
        shard = np.zeros((VP, E), np.float32)
        shard[:VS] = emb[c * VS:(c + 1) * VS]
        nrm = np.maximum(np.linalg.norm(shard, axis=1, keepdims=True), 1e-8)
        emb_n = shard / nrm
        # et2: [jj, 128e, 2, VP] with component i = e-chunk (2*jj+i)
        et = (emb_n.T * N_SCALE).reshape(2, 2, 128, VP).transpose(0, 2, 1, 3)
        et2 = np.ascontiguousarray(_q8(et).reshape(2, 128, 2 * VP))
        # en2: [128v, KK, 2, E] with component i = v-tile (2*kk+i)
        en = (shard * W_SCALE).reshape(KK, 2, 128, E).transpose(2, 0, 1, 3)
        en2 = np.ascontiguousarray(_q8(en).reshape(128, KK * 2 * E))
        # correction stationary: KL[f, e] = q8(20 dbar S^T), S = shard^T emb_n
        S = shard.T @ emb_n                    # [e, f]
        kl = _q8(20.0 * DBAR_R * S.T)          # [f, e]
        k2 = np.ascontiguousarray(
            kl.reshape(2, 2, 128, E).transpose(0, 2, 1, 3).reshape(2, 128, 2 * E)
        )
        in_maps.append({
            "a82": a82,
            "pw82": pw82,
            "proj_b": pb16,
            "et2": et2,
            "en2": en2,
            "k2": k2,
            "onesv": onesv,
            "ident": ident,
        })
    return in_maps


def kernel(audio_feat, proj_w, proj_b, token_embedding, _trace=False):
    nc = _get_program()
    in_maps = _prep_in_maps(audio_feat, proj_w, proj_b, token_embedding)
    res = run_bass_kernel_spmd(
        nc, in_maps, core_ids=list(range(N_CORES)), trace=_trace
    )
    pe = np.zeros((E, M), np.float64)
    dn = np.zeros((1, M), np.float64)
    for c in range(N_CORES):
        pe += res.results[c]["out_pe"]
        dn += res.results[c]["out_d"]
    out = (pe / W_SCALE / dn).T.reshape(B, N, E).astype(np.float32)
    if _trace:
        return out, res
    return out


# revision 66
# speedup vs baseline: 1.1289x; 1.0019x over previous
"""VQ codebook kernel (nn_KW_CascadedBranch) for 8 Trainium2 NeuronCores.

Reference computation:
    kw   = audio_feat @ proj_w + proj_b                  [B,N,512]
    cos  = normalize(kw) @ normalize(token_embedding).T  [B,N,V]
    p    = softmax(cos / 0.1)
    out  = p @ token_embedding                           [B,N,512]

Strategy: tensor-parallel over the vocab dim V=49408. Each core owns a
6176-row shard (padded to 6400 = 50*128 = 25 DoubleRow pairs), keeps both
embedding layouts resident in SBUF as fp8e4, and computes the partial
(p @ emb) plus the partial softmax denominator for ALL B*N=2048 slots.
Softmax needs no max subtraction: logits = 10*cos are in [-10,10].
Host combines the 8 partials: out = (sum_c pe_c)/64 / (sum_c d_c).

The two big GEMMs (cos-scores, prob@emb) run on the PE in fp8e4 with
MatmulPerfMode.DoubleRow: two 128-deep contraction slices per instruction
at 0.5 cycles/row, i.e. 4x the fp32r rate. Quantization scales are powers
of two folded into the exp scale and the host epilogue:
  kwn*32, emb_n*32 -> scores_psum = 1024*cos, exp scale = 10/1024,
  emb*64           -> out_pe = 64*numerator.
The projection runs in bf16 (its quantization noise is amplified ~40x by
the flat-softmax cancellation, so fp8 is not enough there), with proj_b
folded in as a rank-1 row of the same PSUM accumulation group. Embedding
row norms are host-precomputed weight prep; vocab-pad rows are zeros so
exp(0)=1 there, and the denominator matmul uses a masked ones stationary
(>=32 columns, a dual-fp8 ldweights requirement) to exclude them exactly.

fp8 error feedback: out is a near-cancelling average over ~40k vocab rows
(|out|_rms ~ sigma_emb/200), so the kwn fp8 rounding error delta couples
through J = 10*Cov_p(emb, emb_n) into an output error ~40x larger than
naive estimates. Softmax here is nearly flat (den/VS ~= exp(T^-2/(2*512))
uniformly, +-0.7% over m), so J is well approximated with flat weights:
delta_num ~= dbar*10*S^T@delta, with S = sum_shard emb emb_n^T a host
constant and dbar the spec-derived density constant. The kernel captures
delta (the fp8 rounding residual, written straight to fp8 during the
transpose copy) and adds q8(20*dbar*S^T) @ q8(res32) into the same kwacc
PSUM accumulation group: 2 extra DoubleRow matmuls per (j, mc).
Validated in numpy + HW: maxrel 3.4e-2 -> 8.2e-3.

Scheduling: engine queues are in-order, so the 16-m-tile projection
prologue is split: tiles 0-3 run up front in a 4-bank pipelined scope
(closed before the main pools open), tiles 4-15 are emitted inside the
mc0/mc1 kk loops through a single shared PSUM bank, filling the PE's
slack under the ACT-bound exp stream. GEMM1 for iteration kk+1 issues
ahead of GEMM2(kk) so the PE queue never waits on the exps.
"""

import numpy as np
import ml_dtypes

import concourse.bass as bass
import concourse.mybir as mybir
from concourse import tile
from concourse.bass_utils import run_bass_kernel_spmd

F32 = mybir.dt.float32
F8 = mybir.dt.float8e4
BF16 = mybir.dt.bfloat16
AF = mybir.ActivationFunctionType
OP = mybir.AluOpType
PM = mybir.MatmulPerfMode
NPF8 = ml_dtypes.float8_e4m3
NPBF = ml_dtypes.bfloat16

N_CORES = 8
B, N, D, E, V = 256, 8, 768, 512, 49408
M = B * N                      # 2048 keyword slots
VS = V // N_CORES              # 6176 real vocab rows per core
VT = 50                        # v-tiles of 128 per core (6400 rows, 224 pad)
VP = VT * 128
KK = VT // 2                   # 25 DoubleRow v-tile pairs
MC = 512                       # m-chunk (columns per PSUM accumulator)
NMC = M // MC                  # 4
MT = M // 128                  # 16 m-tiles in the projection prologue
DT = D // 128                  # 6 d-chunks
DD = D // 256                  # 3 DoubleRow d-chunk pairs
EC = E // 128                  # 4 e-chunks (2 DoubleRow pairs)
EXP_SCALE = 10.0 / 1024.0      # 1/T divided by the 32*32 quant scales
W_SCALE = 64.0                 # emb quant scale
N_SCALE = 32.0                 # kwn / emb_n quant scale
DBAR_R = float(np.exp(100.0 / 1024.0))  # E[den]/VS for unit-norm randn data

# (mc, kk) -> prologue m-tile emitted at that point of the main loop
PRO_SCHED = {}
UPFRONT = 16


def _split_multiwait_ctrl(nc, max_waits: int = 1) -> int:
    """This container's walrus rejects instructions carrying more than one
    semaphore wait (CTRL and S3_LW encodings alike). Hoist overflow waits
    onto same-engine NoOps inserted immediately before the offender."""
    n_split = 0
    for fn in nc.m.functions:
        for bb in fn.blocks:
            rebuilt, changed = [], False
            for ins in bb.instructions:
                si = ins.sync_info
                if (
                    si is not None
                    and si.on_wait
                    and len(si.on_wait) > max_waits
                ):
                    waits = list(si.on_wait)
                    head, tail = waits[:-max_waits], waits[-max_waits:]
                    for i in range(0, len(head), max_waits):
                        nop = mybir.InstNoOp(name=f"{ins.name}-ws{i}", ins=[], outs=[])
                        nop.engine = ins.engine
                        nop.sync_info = mybir.SyncInfo(
                            on_wait=head[i:i + max_waits], on_update=[]
                        )
                        rebuilt.append(nop)
                    ins.sync_info = mybir.SyncInfo(
                        on_wait=tail, on_update=list(si.on_update or [])
                    )
                    changed = True
                    n_split += 1
                rebuilt.append(ins)
            if changed:
                bb.instructions = rebuilt
    return n_split


def _pair(ap2d, width):
    """View a flat [128, 2*width] AP as [128, 2, width]."""
    return ap2d.rearrange("p (a w) -> p a w", a=2, w=width)


def build_program():
    nc = bass.Bass(target_bir_lowering=False)

    a82 = nc.dram_tensor("a82", [2 * DD, 128, 2 * M], F8, kind="ExternalInput")
    pw82 = nc.dram_tensor("pw82", [2 * DD, 128, 2 * E], F8, kind="ExternalInput")
    proj_b = nc.dram_tensor("proj_b", [1, E], BF16, kind="ExternalInput")
    et2 = nc.dram_tensor("et2", [2, 128, 2 * VP], F8, kind="ExternalInput")
    en2 = nc.dram_tensor("en2", [128, KK * 2 * E], F8, kind="ExternalInput")
    k2 = nc.dram_tensor("k2", [2, 128, 2 * E], F8, kind="ExternalInput")
    onesv = nc.dram_tensor("onesv", [128, 128], F8, kind="ExternalInput")
    ident = nc.dram_tensor("ident", [128, 128], BF16, kind="ExternalInput")

    out_pe = nc.dram_tensor("out_pe", [E, M], F32, kind="ExternalOutput")
    out_d = nc.dram_tensor("out_d", [1, M], F32, kind="ExternalOutput")

    with tile.TileContext(nc) as tc:
        with (
            tc.tile_pool(name="resident", bufs=1) as res,
            tc.tile_pool(name="small", bufs=1) as small,
            tc.tile_pool(name="prok", bufs=4) as prok,
        ):
            # ---- resident SBUF tiles; DMAs ordered so compute starts early:
            # pw+audio chunks (prologue) -> et2 slices (GEMM1) -> en2 (GEMM2)
            pw8f = res.tile([128, 2 * DD * 2 * E], F8, tag="pw8f")
            pb_sb = small.tile([1, E], BF16, tag="pb")
            id_sb = small.tile([128, 128], BF16, tag="ident")
            a8f = res.tile([128, 2 * DD * 2 * M], F8, tag="a8f")
            et_all = res.tile([128, 4 * VP], F8, tag="et_all")
            ones_sb = small.tile([128, 128], F8, tag="ones_sb")
            en_sb = res.tile([128, KK * 2 * E], F8, tag="en")
            k_all = res.tile([128, 4 * E], F8, tag="k_all")

            et3 = [
                _pair(et_all[:, j * 2 * VP:(j + 1) * 2 * VP], VP) for j in range(2)
            ]
            etd = [_pair(et2[j], VP) for j in range(2)]
            k4 = k_all[:].rearrange("p (j x) -> p j x", j=2, x=2 * E)
            k4s = k2[:].rearrange("j p x -> p j x")
            # audio/pw hi-lo group views: g in 0..2 = hi d-pairs, 3..5 = lo
            av = [_pair(a8f[:, g * 2 * M:(g + 1) * 2 * M], M) for g in range(2 * DD)]
            avs = [_pair(a82[g], M) for g in range(2 * DD)]
            pwv = [_pair(pw8f[:, g * 2 * E:(g + 1) * 2 * E], E) for g in range(2 * DD)]
            # interleave the streams, fewest DMAs (each costs ~300ns of queue
            # overhead): audio m-chunk 0 + first et quarter feed the upfront
            # prologue and GEMM1(kk=0); the correction stationary and the
            # first en piece land before the kk=0 GEMM2 group opens; the
            # trailing et quarters and en pieces arrive mid-loop (mc0's
            # deferred GEMM2 sweep tolerates late en)
            QW = VP // 4
            ENP = (KK * 2 * E) // 5
            nc.sync.dma_start(
                pw8f[:].rearrange("p (g x) -> p g x", g=2 * DD, x=2 * E),
                pw82[:].rearrange("g p x -> p g x"),
            )
            nc.sync.dma_start(pb_sb[:], proj_b[:])
            nc.sync.dma_start(id_sb[:], ident[:])

            def a_dma(lo, hi):
                for g in range(2 * DD):
                    nc.sync.dma_start(
                        av[g][:, :, lo * 128:hi * 128], avs[g][:, :, lo * 128:hi * 128]
                    )

            a_dma(0, 4)
            for j in range(2):
                nc.sync.dma_start(et3[j][:, :, 0:QW], etd[j][:, :, 0:QW])
            nc.sync.dma_start(ones_sb[:], onesv[:])
            nc.sync.dma_start(k4[:], k4s[:])
            nc.sync.dma_start(en_sb[:, 0:ENP], en2[:, 0:ENP])
            a_dma(4, 8)
            for j in range(2):
                nc.sync.dma_start(et3[j][:, :, QW:2 * QW], etd[j][:, :, QW:2 * QW])
            a_dma(8, 12)
            nc.sync.dma_start(en_sb[:, ENP:2 * ENP], en2[:, ENP:2 * ENP])
            for j in range(2):
                nc.sync.dma_start(et3[j][:, :, 2 * QW:3 * QW], etd[j][:, :, 2 * QW:3 * QW])
            a_dma(12, 16)
            for j in range(2):
                nc.sync.dma_start(et3[j][:, :, 3 * QW:4 * QW], etd[j][:, :, 3 * QW:4 * QW])
            for pc in range(2, 5):
                sl = slice(pc * ENP, (pc + 1) * ENP)
                nc.sync.dma_start(en_sb[:, sl], en2[:, sl])
            kwnT = [
                [
                    res.tile([128, 2 * MC], F8, tag=f"kwnT{j}_{c}", name=f"kwnT{j}_{c}")
                    for c in range(NMC)
                ]
                for j in range(2)
            ]
            res8 = [
                [
                    res.tile([128, 2 * MC], F8, tag=f"res8_{j}_{c}", name=f"res8_{j}_{c}")
                    for c in range(NMC)
                ]
                for j in range(2)
            ]
            ones_row = small.tile([1, 128], BF16, tag="ones_row")
            nc.vector.memset(ones_row[:], 1.0)

            kwnT3 = [[_pair(kwnT[j][c][:], MC) for c in range(NMC)] for j in range(2)]
            res8_3 = [[_pair(res8[j][c][:], MC) for c in range(NMC)] for j in range(2)]
            k3 = [_pair(k_all[:, j * 2 * E:(j + 1) * 2 * E], E) for j in range(2)]
            ones3 = _pair(ones_sb[:], 64)

            MAGIC = 0x5F3759DF

            def pro_tile(i, kw_alloc, tp_alloc, act_norm):
                """Projection + normalize + transpose + fp8/residual capture
                for m-tile i. kw_alloc/tp_alloc hand out PSUM tiles. The
                upfront tiles use ACT Square+Sqrt (ACT is idle at start); the
                in-loop tiles keep ACT exp-only and compute 32*rsqrt on DVE
                via the 0x5f3759df bit trick + two Newton steps."""
                kw_ps = kw_alloc(i)
                # 3-pass fp8 DoubleRow projection: hi.hi + lo.hi + hi.lo with
                # residuals stored at the SAME scale (fp8 subnormals carry
                # them), so all passes accumulate into one PSUM group
                msl = slice(i * 128, (i + 1) * 128)
                passes = [(dd, dd) for dd in range(DD)] + \
                         [(DD + dd, dd) for dd in range(DD)] + \
                         [(dd, DD + dd) for dd in range(DD)]
                for n, (ga, gw) in enumerate(passes):
                    nc.tensor.matmul(
                        kw_ps[:],
                        av[ga][:, :, msl],
                        pwv[gw],
                        start=(n == 0), stop=False,
                        perf_mode=PM.DoubleRow,
                    )
                nc.tensor.matmul(
                    kw_ps[:], ones_row[:], pb_sb[:], start=False, stop=True
                )
                sq = prok.tile([128, E], F32, tag="sq_kw", name=f"sq{i}")
                nsq = prok.tile([128, 1], F32, tag="nsq_kw", name=f"nsq{i}")
                tk = prok.tile([128, 1], F32, tag="tk", name=f"tk{i}")
                kw_src = kw_ps
                if act_norm:
                    nc.scalar.activation(sq[:], kw_ps[:], AF.Square, accum_out=nsq[:])
                    sk = prok.tile([128, 1], F32, tag="sk", name=f"sk{i}")
                    nc.scalar.activation(sk[:], nsq[:], AF.Sqrt)
                    rk = prok.tile([128, 1], F32, tag="rk", name=f"rk{i}")
                    nc.vector.reciprocal(rk[:], sk[:])
                    nc.vector.tensor_mul(tk[:], rk[:], rk[:])
                    nc.vector.tensor_mul(tk[:], tk[:], nsq[:])
                    nc.vector.tensor_scalar(tk[:], tk[:], -16.0, 48.0, OP.mult, OP.add)
                    nc.vector.tensor_mul(tk[:], tk[:], rk[:])
                else:
                    # DVE can read only one non-scalar input from PSUM: stage
                    # kw in SBUF (also frees the prologue PSUM bank earlier)
                    kwc = prok.tile([128, E], F32, tag="kwc", name=f"kwc{i}")
                    nc.vector.tensor_copy(kwc[:], kw_ps[:])
                    nc.vector.tensor_mul(sq[:], kwc[:], kwc[:])
                    nc.vector.tensor_reduce(nsq[:], sq[:], mybir.AxisListType.X, OP.add)
                    t1 = prok.tile([128, 1], mybir.dt.int32, tag="t1", name=f"t1_{i}")
                    nc.vector.tensor_scalar(
                        t1[:], nsq[:].bitcast(mybir.dt.int32), 1, 0,
                        OP.logical_shift_right, OP.logical_shift_right,
                    )
                    y0 = prok.tile([128, 1], mybir.dt.int32, tag="y0", name=f"y0_{i}")
                    nc.vector.tensor_scalar(y0[:], t1[:], -1, MAGIC, OP.mult, OP.add)
                    hs = prok.tile([128, 1], F32, tag="hs", name=f"hs{i}")
                    nc.vector.tensor_scalar(hs[:], nsq[:], 0.5, 0.0, OP.mult, OP.add)
                    ya = y0[:].bitcast(F32)
                    aa = prok.tile([128, 1], F32, tag="aa", name=f"aa{i}")
                    cc = prok.tile([128, 1], F32, tag="cc", name=f"cc{i}")
                    y1 = prok.tile([128, 1], F32, tag="y1", name=f"y1_{i}")
                    nc.vector.tensor_mul(aa[:], ya, ya)
                    nc.vector.tensor_mul(aa[:], aa[:], hs[:])
                    nc.vector.tensor_scalar(cc[:], aa[:], -1.0, 1.5, OP.mult, OP.add)
                    nc.vector.tensor_mul(y1[:], ya, cc[:])
                    nc.vector.tensor_mul(aa[:], y1[:], y1[:])
                    nc.vector.tensor_mul(aa[:], aa[:], hs[:])
                    nc.vector.tensor_scalar(cc[:], aa[:], -32.0, 48.0, OP.mult, OP.add)
                    nc.vector.tensor_mul(tk[:], y1[:], cc[:])
                    kw_src = kwc
                kwn = prok.tile([128, E], BF16, tag="kwn", name=f"kwn{i}")
                nc.vector.tensor_scalar_mul(kwn[:], kw_src[:], tk[:])
                for j in range(EC):
                    tpv = tp_alloc(i, j)
                    nc.tensor.transpose(tpv, kwn[:, j * 128:(j + 1) * 128], id_sb[:])
                    c2 = (j % 2) * MC + (i % 4) * 128
                    kpiece = kwnT[j // 2][i // 4][:, c2:c2 + 128]
                    nc.vector.tensor_copy(kpiece, tpv)
                    # fp8 rounding residual (32-scale) for the J-correction
                    nc.vector.tensor_sub(
                        res8[j // 2][i // 4][:, c2:c2 + 128], tpv, kpiece
                    )

            # ---- upfront prologue: m-tiles 0..3 in a pipelined 4-bank scope
            with (
                tc.tile_pool(name="pro_ps", bufs=2, space="PSUM") as pro_ps,
                tc.tile_pool(name="pro_ps2", bufs=2, space="PSUM") as pro_ps2,
            ):
                def kw_up(i):
                    return pro_ps.tile([128, E], F32, tag="kw_ps", name=f"kwps{i}")

                def tp_up(i, j):
                    t = pro_ps2.tile([128, 128], BF16, tag="tp", name=f"tp{i}_{j}")
                    return t[:]

                for i in range(UPFRONT):
                    pro_tile(i, kw_up, tp_up, act_norm=(i < 6))

            # ---- main loop. m-tiles 4..15 stream in through one PSUM bank
            # during mc0/mc1; that bank becomes a third score buffer for
            # mc2/mc3 (the exp->GEMM1 bank-recycle latency costs ~15% of the
            # exp pace at depth 2).
            with (
                tc.tile_pool(name="acc_ps", bufs=4, space="PSUM") as acc_ps,
                tc.tile_pool(name="d_ps", bufs=1, space="PSUM") as d_ps,
                tc.tile_pool(name="pp", bufs=58) as pp,
                tc.tile_pool(name="ob", bufs=5) as ob,
            ):
                def run_mc(mc, sc_ps, defer_gemm2):
                    def gemm1(kk):
                        tiles = []
                        for t in range(2):
                            k = 2 * kk + t
                            sc = sc_ps.tile([128, MC], F32, tag="sc", name=f"sc{kk}_{t}")
                            for j in range(2):
                                nc.tensor.matmul(
                                    sc[:],
                                    et3[j][:, :, k * 128:(k + 1) * 128],
                                    kwnT3[j][mc],
                                    start=(j == 0), stop=(j == 1),
                                    perf_mode=PM.DoubleRow,
                                )
                            tiles.append(sc)
                        return tiles

                    m0 = mc * MC
                    dacc = d_ps.tile([32, MC], F32, tag="dacc")

                    def gemm2_group(kk, p3, kwacc):
                        if kk == 0:
                            # J-correction opens each kwacc group (residuals
                            # are ready before the mc starts; PSUM addition is
                            # order-free), so the mc tail is just the last
                            # GEMM2 + the output copies
                            for j in range(EC):
                                for jj in range(2):
                                    nc.tensor.matmul(
                                        kwacc[j][:],
                                        k3[jj][:, :, j * 128:(j + 1) * 128],
                                        res8_3[jj][mc],
                                        start=(jj == 0), stop=False,
                                        perf_mode=PM.DoubleRow,
                                    )
                        en_kk = _pair(en_sb[:, kk * 2 * E:(kk + 1) * 2 * E], E)
                        for j in range(EC):
                            nc.tensor.matmul(
                                kwacc[j][:],
                                en_kk[:, :, j * 128:(j + 1) * 128],
                                p3,
                                start=False, stop=(kk == KK - 1),
                                perf_mode=PM.DoubleRow,
                            )

                    kwacc = None
                    if not defer_gemm2:
                        kwacc = [
                            acc_ps.tile([128, MC], F32, tag="kwacc", name=f"kwacc{j}")
                            for j in range(EC)
                        ]
                    saved = []
                    cur = gemm1(0)
                    for kk in range(KK):
                        p2 = pp.tile([128, 2 * MC], F8, tag="p2")
                        p3 = _pair(p2[:], MC)
                        for t in range(2):
                            nc.scalar.activation(
                                p2[:, t * MC:(t + 1) * MC], cur[t][:], AF.Exp,
                                scale=EXP_SCALE,
                            )
                        # issue next iteration's score GEMMs ahead of GEMM2 so
                        # the in-order PE queue never stalls behind the exps
                        if kk + 1 < KK:
                            cur = gemm1(kk + 1)
                        if defer_gemm2:
                            saved.append(p3)
                        else:
                            gemm2_group(kk, p3, kwacc)
                        sel = 32 if kk == KK - 1 else 0
                        nc.tensor.matmul(
                            dacc[:],
                            ones3[:, :, sel:sel + 32],
                            p3,
                            start=(kk == 0), stop=(kk == KK - 1),
                            perf_mode=PM.DoubleRow,
                        )
                    dsb = ob.tile([1, MC], F32, tag="dsb")
                    nc.vector.tensor_copy(dsb[:], dacc[0:1, :])
                    nc.sync.dma_start(out_d[:, m0:m0 + MC], dsb[:])
                    if defer_gemm2:
                        # mc0's weighted-sum sweep runs here, overlapped with
                        # the next mc, once the prologue's PSUM banks drained
                        kwacc = [
                            acc_ps.tile([128, MC], F32, tag="kwacc", name=f"kwacc{j}")
                            for j in range(EC)
                        ]
                        for kk in range(KK):
                            gemm2_group(kk, saved[kk], kwacc)
                    for j in range(EC):
                        osb = ob.tile([128, MC], F32, tag="osb")
                        nc.vector.tensor_copy(osb[:], kwacc[j][:])
                        nc.sync.dma_start(
                            out_pe[j * 128:(j + 1) * 128, m0:m0 + MC], osb[:]
                        )

                with tc.tile_pool(name="sc_all", bufs=3, space="PSUM") as sc_all:
                    for mc in range(NMC):
                        run_mc(mc, sc_all, defer_gemm2=(mc <= 1))
    return nc


_CACHED = {}


def _get_program():
    if "nc" not in _CACHED:
        nc = build_program()
        _split_multiwait_ctrl(nc)
        _CACHED["nc"] = nc
    return _CACHED["nc"]


def _q8(x):
    return np.asarray(x, np.float32).astype(NPF8)


def _prep_in_maps(audio_feat, proj_w, proj_b, token_embedding):
    audio = np.asarray(audio_feat, np.float32).reshape(M, D)
    pw = np.asarray(proj_w, np.float32)
    pb = np.asarray(proj_b, np.float32).reshape(1, E)
    emb = np.asarray(token_embedding, np.float32)

    def _hilo_pairs(x, scale):
        """[D, X] -> fp8 hi/lo stacked [2*DD, 128, 2*X] in DoubleRow d-pair
        layout; lo = same-scale residual (lives in fp8 subnormal range)."""
        xs = x * scale
        hi = _q8(xs)
        lo = _q8(xs - hi.astype(np.float32))
        out = []
        for h in (hi, lo):
            out.append(h.reshape(DD, 2, 128, -1).transpose(0, 2, 1, 3))
        return np.ascontiguousarray(
            np.concatenate(out, 0).reshape(2 * DD, 128, -1)
        )

    a82 = _hilo_pairs(audio.T, 1.0)
    pw82 = _hilo_pairs(pw, W_SCALE)
    pb16 = (pb * W_SCALE).astype(NPBF)
    ident = np.eye(128, dtype=np.float32).astype(NPBF)
    # masked ones columns for the denominator matmul (dual-fp8 ldweights
    # needs >=32 stationary columns): [128, 2, 64] -> cols 0:32 all-real
    # pair, cols 32:64 last pair (tile 48 rows 0..31 real, tile 49 pad)
    nreal_last = VS - (VT - 2) * 128          # 32 real rows in tile 48
    onesv = np.zeros((128, 2, 64), np.float32)
    onesv[:, :, 0:32] = 1.0
    onesv[:nreal_last, 0, 32:64] = 1.0
    onesv = _q8(onesv.reshape(128, 128))

    in_maps = []
    for c in range(N_CORES):
        shard = np.zeros((VP, E), np.float32)
        shard[:VS] = emb[c * VS:(c + 1) * VS]
        nrm = np.maximum(np.linalg.norm(shard, axis=1, keepdims=True), 1e-8)
        emb_n = shard / nrm
        # et2: [jj, 128e, 2, VP] with component i = e-chunk (2*jj+i)
        et = (emb_n.T * N_SCALE).reshape(2, 2, 128, VP).transpose(0, 2, 1, 3)
        et2 = np.ascontiguousarray(_q8(et).reshape(2, 128, 2 * VP))
        # en2: [128v, KK, 2, E] with component i = v-tile (2*kk+i)
        en = (shard * W_SCALE).reshape(KK, 2, 128, E).transpose(2, 0, 1, 3)
        en2 = np.ascontiguousarray(_q8(en).reshape(128, KK * 2 * E))
        # correction stationary: KL[f, e] = q8(20 dbar S^T), S = shard^T emb_n
        S = shard.T @ emb_n                    # [e, f]
        kl = _q8(20.0 * DBAR_R * S.T)          # [f, e]
        k2 = np.ascontiguousarray(
            kl.reshape(2, 2, 128, E).transpose(0, 2, 1, 3).reshape(2, 128, 2 * E)
        )
        in_maps.append({
            "a82": a82,
            "pw82": pw82,
            "proj_b": pb16,
            "et2": et2,
            "en2": en2,
            "k2": k2,
            "onesv": onesv,
            "ident": ident,
        })
    return in_maps


def kernel(audio_feat, proj_w, proj_b, token_embedding, _trace=False):
    nc = _get_program()
    in_maps = _prep_in_maps(audio_feat, proj_w, proj_b, token_embedding)
    res = run_bass_kernel_spmd(
        nc, in_maps, core_ids=list(range(N_CORES)), trace=_trace
    )
    pe = np.zeros((E, M), np.float64)
    dn = np.zeros((1, M), np.float64)
    for c in range(N_CORES):
        pe += res.results[c]["out_pe"]
        dn += res.results[c]["out_d"]
    out = (pe / W_SCALE / dn).T.reshape(B, N, E).astype(np.float32)
    if _trace:
        return out, res
    return out
